# revision 11
# baseline (speedup 1.0000x reference)
"""Trainium2 Bass kernel for strided-conv-as-linear (nn_ConvNd_60851096649851).

Computation (see reference): x [B,1024,1024] f32, weight [16,256] f32.
16x16 windows at stride 8 -> 127x127 patches; per patch y = W @ flat(window)
(16 outputs), reshaped to a 4x4 tile of the [B,508,508] output.

Strategy: data-parallel over batch (4 images per core, 8 cores).

Per image: 8 DISJOINT 128-row window tiles (x is read from HBM exactly
once). Rows live on SBUF partitions (natural layout). For each window
tile, out[(i_l,o), j] = sum_kw Wband_kw[row, (i_l,o)]^T @ x[row, 8j+kw]
where Wband_kw is the banded weight (nonzero at row = 8*i_l + kh): 16
accumulating float32r matmuls, K=128, M=128 (i_l in [0,8) x 16 outputs
per half; two halves cover 16 patch slots/tile). Four tiles are batched
in the moving dim (N = 4*128 = 512) so float32r streams 1 cycle/column.

The 16th patch slot of each tile (i = 16t+15, rows 128t+120..128t+135)
crosses the tile boundary: its kh 0..7 rows live in tile t (handled by
the main h=1 matmul band), its kh 8..15 rows are the first 8 rows of
tile t+1. Those remainders are computed by ONE batched boundary matmul
chain (K=7 bands x 8 rows=56, M=7 bands x 16 outs=112, N=4 images x 128)
from a small re-read [56,1032] boundary tile per image; the host adds
the two partial results for those 7 patch rows per image.

All DMA rides the HWDGE rings (the original baseline put ~70 dma_starts
on the gpsimd SWDGE path whose Q7 descriptor emission at ~1.4us each
dominated the runtime): x loads on nc.sync (SP) as one uniform-window
access pattern per 4-tile group, weights + output stores on nc.scalar
(ACT). Outputs are stored as bf16 (halves store traffic; rel err ~2e-3
<< 2e-2 gate). Host prepares banded weights and unscrambles the device
layout into [B,508,508] f32.
"""

import os
import sys

sys.path.insert(0, "/opt/trn_rl_repo")
os.environ.setdefault("JAX_PLATFORMS", "cpu")

import numpy as np

import concourse.bass as bass  # noqa: F401
import concourse.tile as tile
from concourse import bacc, mybir
from concourse.ap import AP
from concourse.bass_utils import run_bass_kernel_spmd

N_CORES = 8
KH = KW = 16
STRIDE = 8
D0 = D1 = 4  # per-patch output tile
OC = 16  # outputs per patch = D0*D1
SLOTS = 16  # patch slots per 128-row tile (slot 15 is split across tiles)
GROUP = 4  # window tiles batched per matmul (moving dim)

_MM_DTYPE = mybir.dt.bfloat16
_OUT_DTYPE = mybir.dt.bfloat16


def build_wband(weight):
    """Banded weights: [128, KW*2*128] f32.

    wb[p, kw, h, m] = W[o, kh*16+kw] where p = 8*(i_l+8h)+kh, m = i_l*16+o,
    i_l in [0,8). Slot i_l+8h==15 keeps only its kh<8 rows (p<128); the
    kh>=8 remainder comes from the boundary matmul.
    """
    W4 = np.asarray(weight, np.float32).reshape(OC, KH, KW)
    wb = np.zeros((128, KW, 2, 128), np.float32)
    for h in range(2):
        for il in range(8):
            ig = il + 8 * h
            for kh in range(KH):
                p = 8 * ig + kh
                if p >= 128:
                    continue
                wb[p, :, h, il * OC : (il + 1) * OC] = W4[:, kh, :].T
    return np.ascontiguousarray(wb.reshape(128, KW * 2 * 128))


def build_wband_bnd(weight, n_bnd):
    """Boundary band weights: [8*n_bnd, KW*16*n_bnd] f32.

    wbb[8*tau+q, kw, tau*16+o] = W[o, 8+q, kw] -- the kh>=8 remainder of
    split patch tau (tau in 0..n_bnd-1), read against rows 0..7 of tile
    tau+1.
    """
    W4 = np.asarray(weight, np.float32).reshape(OC, KH, KW)
    wbb = np.zeros((8 * n_bnd, KW, OC * n_bnd), np.float32)
    for tau in range(n_bnd):
        for q in range(8):
            wbb[8 * tau + q, :, tau * OC : (tau + 1) * OC] = W4[:, 8 + q, :].T
    return np.ascontiguousarray(wbb.reshape(8 * n_bnd, KW * OC * n_bnd))


def build_nc(n_img, H, W, repeat=1, loop=1):
    """Build the per-core Bass program. Returns compiled nc.

    repeat: python-unrolled copies of the whole per-core computation.
    loop: hardware For_i trip count around those copies (timing only).
    """
    nH = (H - KH) // STRIDE + 1
    nW = (W - KW) // STRIDE + 1
    n_tiles = H // 128  # disjoint tiles
    assert n_tiles % GROUP == 0, (n_tiles, GROUP)
    n_groups = n_tiles // GROUP
    n_bnd = n_tiles - 1  # split patches per image
    KB = 8 * n_bnd  # boundary contraction size
    MB = OC * n_bnd  # boundary output partitions
    nWp = ((nW + 3) // 4) * 4  # fp32r needs even moving size; pad j
    NF = GROUP * nWp  # moving free size per main matmul
    NB = n_img * nWp  # moving free size per boundary matmul
    WS = W + STRIDE  # padded slot width so the padded-j column stays in bounds
    assert NB <= 512 and NF <= 512

    nc = bacc.Bacc(
        "TRN2", target_bir_lowering=False, debug=False, num_devices=N_CORES
    )
    f32 = mybir.dt.float32
    x_d = nc.dram_tensor(
        "x", [n_img * H * W], f32, kind="ExternalInput"
    ).ap()
    wb_d = nc.dram_tensor(
        "wb", [128, KW * 2 * 128], _MM_DTYPE, kind="ExternalInput"
    ).ap()
    wbb_d = nc.dram_tensor(
        "wbb", [KB, KW * MB], _MM_DTYPE, kind="ExternalInput"
    ).ap()
    out_d = nc.dram_tensor(
        "out", [n_img, n_groups, 2, 128, NF], _OUT_DTYPE, kind="ExternalOutput"
    ).ap()
    outb_d = nc.dram_tensor(
        "outb", [MB, NB], _OUT_DTYPE, kind="ExternalOutput"
    ).ap()

    with tile.TileContext(nc) as tc:
        with (
            tc.tile_pool(name="wbp", bufs=1) as wbp,
            tc.tile_pool(name="xp", bufs=6) as xp,
            tc.tile_pool(name="xbp", bufs=1 if repeat == 1 else 2) as xbp,
            tc.tile_pool(name="psp", bufs=6, space="PSUM") as psp,
            tc.tile_pool(name="psbp", bufs=1 if repeat == 1 else 2,
                         space="PSUM") as psbp,
            tc.tile_pool(name="op", bufs=6) as op,
        ):
            wb_sb = wbp.tile([128, KW * 2 * 128], _MM_DTYPE)
            wbb_sb = wbp.tile([KB, KW * MB], _MM_DTYPE)
            # weights ride the ACT ring in chunks so kw=0 matmuls can
            # start while later chunks stream
            for c in range(4):
                nc.scalar.dma_start(
                    wb_sb[:, c * 1024 : (c + 1) * 1024],
                    wb_d[:, c * 1024 : (c + 1) * 1024],
                )
            nc.scalar.dma_start(wbb_sb[:], wbb_d[:])

            def emit_rep(rep):
                xb = xbp.tile([KB, n_img * WS], _MM_DTYPE, name="xb")
                xb3 = xb.rearrange("p (b w) -> p b w", b=n_img)
                for b in range(n_img):
                    xbase = b * H * W
                    xgs = []
                    for g in range(n_groups):
                        xg = xp.tile([128, GROUP * WS], _MM_DTYPE, name="xg")
                        xg3 = xg.rearrange("p (t w) -> p t w", t=GROUP)
                        t0 = g * GROUP
                        # one DMA per 4-tile group: uniform window AP
                        # reading WS=W+8 consecutive elements per row (the
                        # 8-col spill into the next row feeds only the
                        # discarded pad-j column). The very last tile of
                        # the last image would spill past the x buffer, so
                        # it loads separately with an in-bounds pad.
                        nu = GROUP
                        if b == n_img - 1 and t0 + GROUP == n_tiles:
                            nu = GROUP - 1
                        src = AP(
                            x_d.tensor,
                            xbase + 128 * t0 * W,
                            [[W, 128], [128 * W, nu], [1, WS]],
                        )
                        nc.gpsimd.dma_start(xg3[:, 0:nu, :], src)
                        for t in range(nu, GROUP):
                            s = 128 * (t0 + t)
                            src = AP(
                                x_d.tensor, xbase + s * W, [[W, 128], [1, W]]
                            )
                            nc.gpsimd.dma_start(xg3[:, t, 0:W], src)
                            # pad cols: any in-bounds data (values unused)
                            srcp = AP(
                                x_d.tensor, xbase + s * W,
                                [[W, 128], [1, STRIDE]],
                            )
                            nc.gpsimd.dma_start(xg3[:, t, W:WS], srcp)
                        xgs.append(xg3)
                    # boundary bands: rows 128*(tau+1) .. +7, tau in 0..6
                    srcb = AP(
                        x_d.tensor,
                        xbase + 128 * W,
                        [[128 * W, n_bnd], [W, 8], [1, WS]],
                    )
                    nc.gpsimd.dma_start(xb3[:, b, :], srcb)

                    ps = [
                        [
                            psp.tile([128, NF], f32, name=f"ps_{b}_{g}_{h}",
                                     tag="ps")
                            for h in range(2)
                        ]
                        for g in range(n_groups)
                    ]
                    for kw in range(KW):
                        for h in range(2):
                            lhsT = wb_sb[
                                :, (kw * 2 + h) * 128 : (kw * 2 + h) * 128 + 128
                            ]
                            for g in range(n_groups):
                                rhs = xgs[g][
                                    :, :,
                                    kw : kw + STRIDE * (nWp - 1) + 1 : STRIDE,
                                ]
                                nc.tensor.matmul(
                                    ps[g][h][:],
                                    lhsT,
                                    rhs,
                                    start=(kw == 0),
                                    stop=(kw == KW - 1),
                                )
                    for g in range(n_groups):
                        for h in range(2):
                            ob = op.tile([128, NF], _OUT_DTYPE, name="ob")
                            nc.vector.tensor_copy(ob[:], ps[g][h][:])
                            nc.scalar.dma_start(out_d[b, g, h], ob[:])

                # batched boundary remainder over all images
                psb = psbp.tile([MB, NB], f32, name=f"psb_{rep}", tag="psb")
                for kw in range(KW):
                    rhsb = xb3[:, :, kw : kw + STRIDE * (nWp - 1) + 1 : STRIDE]
                    nc.tensor.matmul(
                        psb[:],
                        wbb_sb[:, kw * MB : (kw + 1) * MB],
                        rhsb,
                        start=(kw == 0),
                        stop=(kw == KW - 1),
                    )
                obb = op.tile([MB, NB], _OUT_DTYPE, name="obb")
                nc.vector.tensor_copy(obb[:], psb[:])
                nc.scalar.dma_start(outb_d[:], obb[:])

            if loop > 1:
                with tc.For_i(0, loop):
                    for rep in range(repeat):
                        emit_rep(rep)
            else:
                for rep in range(repeat):
                    emit_rep(rep)
    nc.compile()
    return nc, nH, nW, n_groups


def unscramble(dev_out, dev_outb, nH, nW, n_img):
    """Assemble [n_img, nH*4, nW*4] from the device layouts.

    dev_out  [n_img, n_groups, 2, 128, GROUP*nWp]: main results; split
             patches (i = 16t+15) hold only their kh<8 partial sums.
    dev_outb [16*n_bnd, n_img*nWp]: kh>=8 remainders for split patches.
    """
    n_groups = dev_out.shape[1]
    nWp = ((nW + 3) // 4) * 4
    n_bnd = n_groups * GROUP - 1
    dev = np.asarray(dev_out, np.float32)
    dev = dev.reshape(n_img, n_groups, 2, 8, D0, D1, GROUP, nWp)[..., :nW]
    bnd = np.asarray(dev_outb, np.float32)
    bnd = bnd.reshape(n_bnd, D0, D1, n_img, nWp)[..., :nW]
    out5 = np.empty((n_img, nH, D0, nW, D1), np.float32)
    for g in range(n_groups):
        for t in range(GROUP):
            tau = g * GROUP + t
            for h in range(2):
                for il in range(8):
                    ig = il + 8 * h
                    i = SLOTS * tau + ig
                    if i >= nH:
                        continue
                    # dev[b, g, h, il, d0, d1, t, j] -> out5[b, i, d0, j, d1]
                    v = dev[:, g, h, il, :, :, t, :].transpose(0, 1, 3, 2)
                    if ig == SLOTS - 1:
                        # add the kh>=8 remainder: bnd[tau, d0, d1, b, j]
                        v = v + bnd[tau].transpose(2, 0, 3, 1)
                    out5[:, i] = v
    return out5.reshape(n_img, nH * D0, nW * D1)


def _pjrt_timed_run(nc, in_maps, n_iters=6):
    """Clone of bass2jax.run_bass_via_pjrt's multi-core path, without
    donation, with device-resident inputs, timing each execution.
    Returns (results_per_core, [wall_seconds per iter])."""
    import time

    import jax
    from jax.sharding import Mesh, PartitionSpec
    from jax.experimental.shard_map import shard_map

    from concourse import bass2jax, mybir as _mb
    from concourse.bass2jax import _bass_exec_p, partition_id_tensor

    bass2jax.install_neuronx_cc_hook()
    n_cores = len(in_maps)
    partition_name = nc.partition_id_tensor.name if nc.partition_id_tensor else None

    in_names, out_names, out_avals = [], [], []
    zero_outs = []
    for alloc in nc.m.functions[0].allocations:
        if not isinstance(alloc, _mb.MemoryLocationSet):
            continue
        name = alloc.memorylocations[0].name
        if alloc.kind == "ExternalInput":
            if name != partition_name:
                in_names.append(name)
        elif alloc.kind == "ExternalOutput":
            shape = tuple(alloc.tensor_shape)
            dtype = _mb.dt.np(alloc.dtype)
            out_names.append(name)
            out_avals.append(jax.core.ShapedArray(shape, dtype))
            zero_outs.append(np.zeros(shape, dtype))
    n_params = len(in_names)
    in_names_all = in_names + out_names
    if partition_name is not None:
        in_names_all.append(partition_name)

    def _body(*args):
        operands = list(args)
        if partition_name is not None:
            operands.append(partition_id_tensor())
        outs = _bass_exec_p.bind(
            *operands,
            out_avals=tuple(out_avals),
            in_names=tuple(in_names_all),
            out_names=tuple(out_names),
            lowering_input_output_aliases=(),
            sim_require_finite=True,
            sim_require_nnan=True,
            nc=nc,
        )
        return tuple(outs)

    devices = jax.devices()[:n_cores]
    mesh = Mesh(np.asarray(devices), ("core",))
    in_specs = (PartitionSpec("core"),) * (n_params + len(out_names))
    out_specs = (PartitionSpec("core"),) * len(out_names)
    sharded = jax.jit(
        shard_map(_body, mesh=mesh, in_specs=in_specs, out_specs=out_specs,
                  check_rep=False),
        keep_unused=True,
    )
    concat_in = [
        np.concatenate([np.asarray(in_maps[c][n]) for c in range(n_cores)], axis=0)
        for n in in_names
    ]
    concat_zeros = [
        np.zeros((n_cores * z.shape[0], *z.shape[1:]), z.dtype) for z in zero_outs
    ]
    from jax.sharding import NamedSharding

    dev_in = [
        jax.device_put(a, NamedSharding(mesh, PartitionSpec("core")))
        for a in concat_in + concat_zeros
    ]
    out_arrs = sharded(*dev_in)  # warmup + compile
    jax.block_until_ready(out_arrs)
    times = []
    for _ in range(n_iters):
        t0 = time.perf_counter()
        out_arrs = sharded(*dev_in)
        jax.block_until_ready(out_arrs)
        times.append(time.perf_counter() - t0)
    results = [
        {
            n: np.asarray(out_arrs[i]).reshape(n_cores, *out_avals[i].shape)[c]
            for i, n in enumerate(out_names)
        }
        for c in range(n_cores)
    ]
    return results, times


_CACHE = {}


def _get_nc(n_img, H, W, repeat=1, loop=1):
    key = (n_img, H, W, repeat, loop)
    if key not in _CACHE:
        _CACHE[key] = build_nc(n_img, H, W, repeat, loop)
    return _CACHE[key]


def kernel(x, weight, _timed=False, _repeat=1):
    x = np.asarray(x, np.float32)
    weight = np.asarray(weight, np.float32)
    B, H, W = x.shape
    assert B % N_CORES == 0
    n_img = B // N_CORES
    nc, nH, nW, n_groups = _get_nc(n_img, H, W, _repeat)
    import ml_dtypes
    wb = build_wband(weight).astype(ml_dtypes.bfloat16)
    wbb = build_wband_bnd(weight, H // 128 - 1).astype(ml_dtypes.bfloat16)
    in_maps = [
        {
            "x": np.ascontiguousarray(x[c * n_img : (c + 1) * n_img]).reshape(-1),
            "wb": wb,
            "wbb": wbb,
        }
        for c in range(N_CORES)
    ]
    if _timed:
        results, times = _pjrt_timed_run(nc, in_maps)
    else:
        results = run_bass_kernel_spmd(
            nc, in_maps, core_ids=list(range(N_CORES))
        ).results
        times = None
    shards = [
        unscramble(results[c]["out"], results[c]["outb"], nH, nW, n_img)
        for c in range(N_CORES)
    ]
    full = np.concatenate(shards, axis=0)
    if _timed:
        return full, times
    return full


# revision 12
# speedup vs baseline: 1.0124x; 1.0124x over previous
"""Trainium2 Bass kernel for strided-conv-as-linear (nn_ConvNd_60851096649851).

Computation (see reference): x [B,1024,1024] f32, weight [16,256] f32.
16x16 windows at stride 8 -> 127x127 patches; per patch y = W @ flat(window)
(16 outputs), reshaped to a 4x4 tile of the [B,508,508] output.

Strategy: data-parallel over batch (4 images per core, 8 cores).

Per image: 8 DISJOINT 128-row window tiles (x is read from HBM exactly
once). Rows live on SBUF partitions (natural layout). For each window
tile, out[(i_l,o), j] = sum_kw Wband_kw[row, (i_l,o)]^T @ x[row, 8j+kw]
where Wband_kw is the banded weight (nonzero at row = 8*i_l + kh): 16
accumulating float32r matmuls, K=128, M=128 (i_l in [0,8) x 16 outputs
per half; two halves cover 16 patch slots/tile). Four tiles are batched
in the moving dim (N = 4*128 = 512) so float32r streams 1 cycle/column.

The 16th patch slot of each tile (i = 16t+15, rows 128t+120..128t+135)
crosses the tile boundary: its kh 0..7 rows live in tile t (handled by
the main h=1 matmul band), its kh 8..15 rows are the first 8 rows of
tile t+1. Those remainders are computed by ONE batched boundary matmul
chain (K=7 bands x 8 rows=56, M=7 bands x 16 outs=112, N=4 images x 128)
from a small re-read [56,1032] boundary tile per image; the host adds
the two partial results for those 7 patch rows per image.

All DMA rides the HWDGE rings (the original baseline put ~70 dma_starts
on the gpsimd SWDGE path whose Q7 descriptor emission at ~1.4us each
dominated the runtime): x loads on nc.sync (SP) as one uniform-window
access pattern per 4-tile group, weights + output stores on nc.scalar
(ACT). Outputs are stored as bf16 (halves store traffic; rel err ~2e-3
<< 2e-2 gate). Host prepares banded weights and unscrambles the device
layout into [B,508,508] f32.
"""

import os
import sys

sys.path.insert(0, "/opt/trn_rl_repo")
os.environ.setdefault("JAX_PLATFORMS", "cpu")

import numpy as np

import concourse.bass as bass  # noqa: F401
import concourse.tile as tile
from concourse import bacc, mybir
from concourse.ap import AP
from concourse.bass_utils import run_bass_kernel_spmd

N_CORES = 8
KH = KW = 16
STRIDE = 8
D0 = D1 = 4  # per-patch output tile
OC = 16  # outputs per patch = D0*D1
SLOTS = 16  # patch slots per 128-row tile (slot 15 is split across tiles)
GROUP = 4  # window tiles batched per matmul (moving dim)

_MM_DTYPE = mybir.dt.bfloat16
_OUT_DTYPE = mybir.dt.bfloat16


def build_wband(weight):
    """Banded weights: [128, KW*2*128] f32.

    wb[p, kw, h, m] = W[o, kh*16+kw] where p = 8*(i_l+8h)+kh, m = i_l*16+o,
    i_l in [0,8). Slot i_l+8h==15 keeps only its kh<8 rows (p<128); the
    kh>=8 remainder comes from the boundary matmul.
    """
    W4 = np.asarray(weight, np.float32).reshape(OC, KH, KW)
    wb = np.zeros((128, KW, 2, 128), np.float32)
    for h in range(2):
        for il in range(8):
            ig = il + 8 * h
            for kh in range(KH):
                p = 8 * ig + kh
                if p >= 128:
                    continue
                wb[p, :, h, il * OC : (il + 1) * OC] = W4[:, kh, :].T
    return np.ascontiguousarray(wb.reshape(128, KW * 2 * 128))


def build_wband_bnd(weight, n_bnd):
    """Boundary band weights: [8*n_bnd, KW*16*n_bnd] f32.

    wbb[8*tau+q, kw, tau*16+o] = W[o, 8+q, kw] -- the kh>=8 remainder of
    split patch tau (tau in 0..n_bnd-1), read against rows 0..7 of tile
    tau+1.
    """
    W4 = np.asarray(weight, np.float32).reshape(OC, KH, KW)
    wbb = np.zeros((8 * n_bnd, KW, OC * n_bnd), np.float32)
    for tau in range(n_bnd):
        for q in range(8):
            wbb[8 * tau + q, :, tau * OC : (tau + 1) * OC] = W4[:, 8 + q, :].T
    return np.ascontiguousarray(wbb.reshape(8 * n_bnd, KW * OC * n_bnd))


def build_nc(n_img, H, W, repeat=1, loop=1):
    """Build the per-core Bass program. Returns compiled nc.

    repeat: python-unrolled copies of the whole per-core computation.
    loop: hardware For_i trip count around those copies (timing only).
    """
    nH = (H - KH) // STRIDE + 1
    nW = (W - KW) // STRIDE + 1
    n_tiles = H // 128  # disjoint tiles
    assert n_tiles % GROUP == 0, (n_tiles, GROUP)
    n_groups = n_tiles // GROUP
    n_bnd = n_tiles - 1  # split patches per image
    KB = 8 * n_bnd  # boundary contraction size
    MB = OC * n_bnd  # boundary output partitions
    nWp = ((nW + 3) // 4) * 4  # fp32r needs even moving size; pad j
    NF = GROUP * nWp  # moving free size per main matmul
    NB = n_img * nWp  # moving free size per boundary matmul
    WS = W + STRIDE  # padded slot width so the padded-j column stays in bounds
    assert NB <= 512 and NF <= 512

    nc = bacc.Bacc(
        "TRN2", target_bir_lowering=False, debug=False, num_devices=N_CORES
    )
    f32 = mybir.dt.float32
    x_d = nc.dram_tensor(
        "x", [n_img * H * W], _MM_DTYPE, kind="ExternalInput"
    ).ap()
    wb_d = nc.dram_tensor(
        "wb", [128, KW * 2 * 128], _MM_DTYPE, kind="ExternalInput"
    ).ap()
    wbb_d = nc.dram_tensor(
        "wbb", [KB, KW * MB], _MM_DTYPE, kind="ExternalInput"
    ).ap()
    out_d = nc.dram_tensor(
        "out", [n_img, n_groups, 2, 128, NF], _OUT_DTYPE, kind="ExternalOutput"
    ).ap()
    outb_d = nc.dram_tensor(
        "outb", [MB, NB], _OUT_DTYPE, kind="ExternalOutput"
    ).ap()

    with tile.TileContext(nc) as tc:
        with (
            tc.tile_pool(name="wbp", bufs=1) as wbp,
            tc.tile_pool(name="xp", bufs=6) as xp,
            tc.tile_pool(name="xbp", bufs=1 if repeat == 1 else 2) as xbp,
            tc.tile_pool(name="psp", bufs=6, space="PSUM") as psp,
            tc.tile_pool(name="psbp", bufs=1 if repeat == 1 else 2,
                         space="PSUM") as psbp,
            tc.tile_pool(name="op", bufs=6) as op,
        ):
            wb_sb = wbp.tile([128, KW * 2 * 128], _MM_DTYPE)
            wbb_sb = wbp.tile([KB, KW * MB], _MM_DTYPE)
            # weights ride the ACT ring in chunks so kw=0 matmuls can
            # start while later chunks stream
            for c in range(4):
                nc.scalar.dma_start(
                    wb_sb[:, c * 1024 : (c + 1) * 1024],
                    wb_d[:, c * 1024 : (c + 1) * 1024],
                )
            nc.scalar.dma_start(wbb_sb[:], wbb_d[:])

            def emit_rep(rep):
                xb = xbp.tile([KB, n_img * WS], _MM_DTYPE, name="xb")
                xb3 = xb.rearrange("p (b w) -> p b w", b=n_img)
                for b in range(n_img):
                    xbase = b * H * W
                    xgs = []
                    for g in range(n_groups):
                        xg = xp.tile([128, GROUP * WS], _MM_DTYPE, name="xg")
                        xg3 = xg.rearrange("p (t w) -> p t w", t=GROUP)
                        t0 = g * GROUP
                        # one DMA per 4-tile group: uniform window AP
                        # reading WS=W+8 consecutive elements per row (the
                        # 8-col spill into the next row feeds only the
                        # discarded pad-j column). The very last tile of
                        # the last image would spill past the x buffer, so
                        # it loads separately with an in-bounds pad.
                        nu = GROUP
                        if b == n_img - 1 and t0 + GROUP == n_tiles:
                            nu = GROUP - 1
                        src = AP(
                            x_d.tensor,
                            xbase + 128 * t0 * W,
                            [[W, 128], [128 * W, nu], [1, WS]],
                        )
                        nc.sync.dma_start(xg3[:, 0:nu, :], src)
                        for t in range(nu, GROUP):
                            s = 128 * (t0 + t)
                            src = AP(
                                x_d.tensor, xbase + s * W, [[W, 128], [1, W]]
                            )
                            nc.sync.dma_start(xg3[:, t, 0:W], src)
                            # pad cols: any in-bounds data (values unused)
                            srcp = AP(
                                x_d.tensor, xbase + s * W,
                                [[W, 128], [1, STRIDE]],
                            )
                            nc.sync.dma_start(xg3[:, t, W:WS], srcp)
                        xgs.append(xg3)
                    # boundary bands: rows 128*(tau+1) .. +7, tau in 0..6
                    srcb = AP(
                        x_d.tensor,
                        xbase + 128 * W,
                        [[128 * W, n_bnd], [W, 8], [1, WS]],
                    )
                    nc.sync.dma_start(xb3[:, b, :], srcb)

                    ps = [
                        [
                            psp.tile([128, NF], f32, name=f"ps_{b}_{g}_{h}",
                                     tag="ps")
                            for h in range(2)
                        ]
                        for g in range(n_groups)
                    ]
                    for kw in range(KW):
                        for h in range(2):
                            lhsT = wb_sb[
                                :, (kw * 2 + h) * 128 : (kw * 2 + h) * 128 + 128
                            ]
                            for g in range(n_groups):
                                rhs = xgs[g][
                                    :, :,
                                    kw : kw + STRIDE * (nWp - 1) + 1 : STRIDE,
                                ]
                                nc.tensor.matmul(
                                    ps[g][h][:],
                                    lhsT,
                                    rhs,
                                    start=(kw == 0),
                                    stop=(kw == KW - 1),
                                )
                    for g in range(n_groups):
                        for h in range(2):
                            ob = op.tile([128, NF], _OUT_DTYPE, name="ob")
                            nc.vector.tensor_copy(ob[:], ps[g][h][:])
                            nc.scalar.dma_start(out_d[b, g, h], ob[:])

                # batched boundary remainder over all images
                psb = psbp.tile([MB, NB], f32, name=f"psb_{rep}", tag="psb")
                for kw in range(KW):
                    rhsb = xb3[:, :, kw : kw + STRIDE * (nWp - 1) + 1 : STRIDE]
                    nc.tensor.matmul(
                        psb[:],
                        wbb_sb[:, kw * MB : (kw + 1) * MB],
                        rhsb,
                        start=(kw == 0),
                        stop=(kw == KW - 1),
                    )
                obb = op.tile([MB, NB], _OUT_DTYPE, name="obb")
                nc.vector.tensor_copy(obb[:], psb[:])
                nc.scalar.dma_start(outb_d[:], obb[:])

            if loop > 1:
                with tc.For_i(0, loop):
                    for rep in range(repeat):
                        emit_rep(rep)
            else:
                for rep in range(repeat):
                    emit_rep(rep)
    nc.compile()
    return nc, nH, nW, n_groups


def unscramble(dev_out, dev_outb, nH, nW, n_img):
    """Assemble [n_img, nH*4, nW*4] from the device layouts.

    dev_out  [n_img, n_groups, 2, 128, GROUP*nWp]: main results; split
             patches (i = 16t+15) hold only their kh<8 partial sums.
    dev_outb [16*n_bnd, n_img*nWp]: kh>=8 remainders for split patches.
    """
    n_groups = dev_out.shape[1]
    nWp = ((nW + 3) // 4) * 4
    n_bnd = n_groups * GROUP - 1
    dev = np.asarray(dev_out, np.float32)
    dev = dev.reshape(n_img, n_groups, 2, 8, D0, D1, GROUP, nWp)[..., :nW]
    bnd = np.asarray(dev_outb, np.float32)
    bnd = bnd.reshape(n_bnd, D0, D1, n_img, nWp)[..., :nW]
    out5 = np.empty((n_img, nH, D0, nW, D1), np.float32)
    for g in range(n_groups):
        for t in range(GROUP):
            tau = g * GROUP + t
            for h in range(2):
                for il in range(8):
                    ig = il + 8 * h
                    i = SLOTS * tau + ig
                    if i >= nH:
                        continue
                    # dev[b, g, h, il, d0, d1, t, j] -> out5[b, i, d0, j, d1]
                    v = dev[:, g, h, il, :, :, t, :].transpose(0, 1, 3, 2)
                    if ig == SLOTS - 1:
                        # add the kh>=8 remainder: bnd[tau, d0, d1, b, j]
                        v = v + bnd[tau].transpose(2, 0, 3, 1)
                    out5[:, i] = v
    return out5.reshape(n_img, nH * D0, nW * D1)


def _pjrt_timed_run(nc, in_maps, n_iters=6):
    """Clone of bass2jax.run_bass_via_pjrt's multi-core path, without
    donation, with device-resident inputs, timing each execution.
    Returns (results_per_core, [wall_seconds per iter])."""
    import time

    import jax
    from jax.sharding import Mesh, PartitionSpec
    from jax.experimental.shard_map import shard_map

    from concourse import bass2jax, mybir as _mb
    from concourse.bass2jax import _bass_exec_p, partition_id_tensor

    bass2jax.install_neuronx_cc_hook()
    n_cores = len(in_maps)
    partition_name = nc.partition_id_tensor.name if nc.partition_id_tensor else None

    in_names, out_names, out_avals = [], [], []
    zero_outs = []
    for alloc in nc.m.functions[0].allocations:
        if not isinstance(alloc, _mb.MemoryLocationSet):
            continue
        name = alloc.memorylocations[0].name
        if alloc.kind == "ExternalInput":
            if name != partition_name:
                in_names.append(name)
        elif alloc.kind == "ExternalOutput":
            shape = tuple(alloc.tensor_shape)
            dtype = _mb.dt.np(alloc.dtype)
            out_names.append(name)
            out_avals.append(jax.core.ShapedArray(shape, dtype))
            zero_outs.append(np.zeros(shape, dtype))
    n_params = len(in_names)
    in_names_all = in_names + out_names
    if partition_name is not None:
        in_names_all.append(partition_name)

    def _body(*args):
        operands = list(args)
        if partition_name is not None:
            operands.append(partition_id_tensor())
        outs = _bass_exec_p.bind(
            *operands,
            out_avals=tuple(out_avals),
            in_names=tuple(in_names_all),
            out_names=tuple(out_names),
            lowering_input_output_aliases=(),
            sim_require_finite=True,
            sim_require_nnan=True,
            nc=nc,
        )
        return tuple(outs)

    devices = jax.devices()[:n_cores]
    mesh = Mesh(np.asarray(devices), ("core",))
    in_specs = (PartitionSpec("core"),) * (n_params + len(out_names))
    out_specs = (PartitionSpec("core"),) * len(out_names)
    sharded = jax.jit(
        shard_map(_body, mesh=mesh, in_specs=in_specs, out_specs=out_specs,
                  check_rep=False),
        keep_unused=True,
    )
    concat_in = [
        np.concatenate([np.asarray(in_maps[c][n]) for c in range(n_cores)], axis=0)
        for n in in_names
    ]
    concat_zeros = [
        np.zeros((n_cores * z.shape[0], *z.shape[1:]), z.dtype) for z in zero_outs
    ]
    from jax.sharding import NamedSharding

    dev_in = [
        jax.device_put(a, NamedSharding(mesh, PartitionSpec("core")))
        for a in concat_in + concat_zeros
    ]
    out_arrs = sharded(*dev_in)  # warmup + compile
    jax.block_until_ready(out_arrs)
    times = []
    for _ in range(n_iters):
        t0 = time.perf_counter()
        out_arrs = sharded(*dev_in)
        jax.block_until_ready(out_arrs)
        times.append(time.perf_counter() - t0)
    results = [
        {
            n: np.asarray(out_arrs[i]).reshape(n_cores, *out_avals[i].shape)[c]
            for i, n in enumerate(out_names)
        }
        for c in range(n_cores)
    ]
    return results, times


_CACHE = {}


def _get_nc(n_img, H, W, repeat=1, loop=1):
    key = (n_img, H, W, repeat, loop)
    if key not in _CACHE:
        _CACHE[key] = build_nc(n_img, H, W, repeat, loop)
    return _CACHE[key]


def kernel(x, weight, _timed=False, _repeat=1):
    x = np.asarray(x, np.float32)
    weight = np.asarray(weight, np.float32)
    B, H, W = x.shape
    assert B % N_CORES == 0
    n_img = B // N_CORES
    nc, nH, nW, n_groups = _get_nc(n_img, H, W, _repeat)
    import ml_dtypes
    xb16 = x.astype(ml_dtypes.bfloat16)
    wb = build_wband(weight).astype(ml_dtypes.bfloat16)
    wbb = build_wband_bnd(weight, H // 128 - 1).astype(ml_dtypes.bfloat16)
    in_maps = [
        {
            "x": np.ascontiguousarray(xb16[c * n_img : (c + 1) * n_img]).reshape(-1),
            "wb": wb,
            "wbb": wbb,
        }
        for c in range(N_CORES)
    ]
    if _timed:
        results, times = _pjrt_timed_run(nc, in_maps)
    else:
        results = run_bass_kernel_spmd(
            nc, in_maps, core_ids=list(range(N_CORES))
        ).results
        times = None
    shards = [
        unscramble(results[c]["out"], results[c]["outb"], nH, nW, n_img)
        for c in range(N_CORES)
    ]
    full = np.concatenate(shards, axis=0)
    if _timed:
        return full, times
    return full


# revision 14
# speedup vs baseline: 690.5169x; 682.0507x over previous
"""Trainium2 Bass kernel for strided-conv-as-linear (nn_ConvNd_60851096649851).

Computation (see reference): x [B,1024,1024] f32, weight [16,256] f32.
16x16 windows at stride 8 -> 127x127 patches; per patch y = W @ flat(window)
(16 outputs), reshaped to a 4x4 tile of the [B,508,508] output.

Strategy: data-parallel over batch (4 images per core, 8 cores).

Per image: 8 DISJOINT 128-row window tiles (x is read from HBM exactly
once). Rows live on SBUF partitions (natural layout). For each window
tile, out[(i_l,o), j] = sum_kw Wband_kw[row, (i_l,o)]^T @ x[row, 8j+kw]
where Wband_kw is the banded weight (nonzero at row = 8*i_l + kh): 16
accumulating float32r matmuls, K=128, M=128 (i_l in [0,8) x 16 outputs
per half; two halves cover 16 patch slots/tile). Four tiles are batched
in the moving dim (N = 4*128 = 512) so float32r streams 1 cycle/column.

The 16th patch slot of each tile (i = 16t+15, rows 128t+120..128t+135)
crosses the tile boundary: its kh 0..7 rows live in tile t (handled by
the main h=1 matmul band), its kh 8..15 rows are the first 8 rows of
tile t+1. Those remainders are computed by ONE batched boundary matmul
chain (K=7 bands x 8 rows=56, M=7 bands x 16 outs=112, N=4 images x 128)
from a small re-read [56,1032] boundary tile per image; the host adds
the two partial results for those 7 patch rows per image.

All DMA rides the HWDGE rings (the original baseline put ~70 dma_starts
on the gpsimd SWDGE path whose Q7 descriptor emission at ~1.4us each
dominated the runtime): x loads on nc.sync (SP) as one uniform-window
access pattern per 4-tile group, weights + output stores on nc.scalar
(ACT). Outputs are stored as bf16 (halves store traffic; rel err ~2e-3
<< 2e-2 gate). Host prepares banded weights and unscrambles the device
layout into [B,508,508] f32.
"""

import os
import sys

sys.path.insert(0, "/opt/trn_rl_repo")
os.environ.setdefault("JAX_PLATFORMS", "cpu")

import numpy as np

import concourse.bass as bass  # noqa: F401
import concourse.tile as tile
from concourse import bacc, mybir
from concourse.ap import AP
from concourse.bass_utils import run_bass_kernel_spmd

N_CORES = 8
KH = KW = 16
STRIDE = 8
D0 = D1 = 4  # per-patch output tile
OC = 16  # outputs per patch = D0*D1
SLOTS = 16  # patch slots per 128-row tile (slot 15 is split across tiles)
GROUP = 4  # window tiles batched per matmul (moving dim)

_MM_DTYPE = mybir.dt.float32r
_OUT_DTYPE = mybir.dt.bfloat16


def build_wband(weight):
    """Banded weights: [128, KW*2*128] f32.

    wb[p, kw, h, m] = W[o, kh*16+kw] where p = 8*(i_l+8h)+kh, m = i_l*16+o,
    i_l in [0,8). Slot i_l+8h==15 keeps only its kh<8 rows (p<128); the
    kh>=8 remainder comes from the boundary matmul.
    """
    W4 = np.asarray(weight, np.float32).reshape(OC, KH, KW)
    wb = np.zeros((128, KW, 2, 128), np.float32)
    for h in range(2):
        for il in range(8):
            ig = il + 8 * h
            for kh in range(KH):
                p = 8 * ig + kh
                if p >= 128:
                    continue
                wb[p, :, h, il * OC : (il + 1) * OC] = W4[:, kh, :].T
    return np.ascontiguousarray(wb.reshape(128, KW * 2 * 128))


def build_wband_bnd(weight, n_bnd):
    """Boundary band weights: [16*n_bnd, 8*16*n_bnd] f32.

    The kh>=8 remainder of split patch tau (tau in 0..n_bnd-1), read
    against rows 0..7 of tile tau+1. The boundary contraction only needs
    8*n_bnd=56 rows, so TWO copies of the bands are stacked in K (the
    second reads the x bands shifted by one conv stride) and each of the
    8 phase-matmuls covers two kw values at once:
      rows 8*tau+q       : W[o, 8+q, phase]      (x col 8j+phase)
      rows 56 + 8*tau+q  : W[o, 8+q, phase+8]    (x col 8j+8+phase)
    """
    W4 = np.asarray(weight, np.float32).reshape(OC, KH, KW)
    half = 8 * n_bnd
    wbb = np.zeros((2 * half, 8, OC * n_bnd), np.float32)
    for tau in range(n_bnd):
        for q in range(8):
            wbb[8 * tau + q, :, tau * OC : (tau + 1) * OC] = \
                W4[:, 8 + q, 0:8].T
            wbb[half + 8 * tau + q, :, tau * OC : (tau + 1) * OC] = \
                W4[:, 8 + q, 8:16].T
    return np.ascontiguousarray(wbb.reshape(2 * half, 8 * OC * n_bnd))


def build_nc(n_img, H, W, repeat=1, loop=1):
    """Build the per-core Bass program. Returns compiled nc.

    repeat: python-unrolled copies of the whole per-core computation.
    loop: hardware For_i trip count around those copies (timing only).
    """
    nH = (H - KH) // STRIDE + 1
    nW = (W - KW) // STRIDE + 1
    n_tiles = H // 128  # disjoint tiles
    assert n_tiles % GROUP == 0, (n_tiles, GROUP)
    n_groups = n_tiles // GROUP
    n_bnd = n_tiles - 1  # split patches per image
    KB = 16 * n_bnd  # boundary contraction size (two phase-shifted copies)
    MB = OC * n_bnd  # boundary output partitions
    nWp = ((nW + 3) // 4) * 4  # fp32r needs even moving size; pad j
    NF = GROUP * nWp  # moving free size per main matmul
    NB = n_img * nWp  # moving free size per boundary matmul
    WS = W + STRIDE  # padded slot width so the padded-j column stays in bounds
    assert NB <= 512 and NF <= 512

    nc = bacc.Bacc(
        "TRN2", target_bir_lowering=False, debug=False, num_devices=N_CORES
    )
    f32 = mybir.dt.float32
    x_d = nc.dram_tensor(
        "x", [n_img * H * W], _MM_DTYPE, kind="ExternalInput"
    ).ap()
    wb_d = nc.dram_tensor(
        "wb", [128, KW * 2 * 128], _MM_DTYPE, kind="ExternalInput"
    ).ap()
    wbb_d = nc.dram_tensor(
        "wbb", [KB, 8 * MB], _MM_DTYPE, kind="ExternalInput"
    ).ap()
    out_d = nc.dram_tensor(
        "out", [n_img, n_groups, 2, 128, NF], _OUT_DTYPE, kind="ExternalOutput"
    ).ap()
    outb_d = nc.dram_tensor(
        "outb", [MB, NB], _OUT_DTYPE, kind="ExternalOutput"
    ).ap()

    with tile.TileContext(nc) as tc:
        with (
            tc.tile_pool(name="wbp", bufs=1) as wbp,
            tc.tile_pool(name="xp", bufs=6) as xp,
            tc.tile_pool(name="xbp", bufs=1 if repeat == 1 else 2) as xbp,
            tc.tile_pool(name="psp", bufs=6, space="PSUM") as psp,
            tc.tile_pool(name="psbp", bufs=1 if repeat == 1 else 2,
                         space="PSUM") as psbp,
            tc.tile_pool(name="op", bufs=6) as op,
        ):
            wb_sb = wbp.tile([128, KW * 2 * 128], _MM_DTYPE)
            wbb_sb = wbp.tile([KB, 8 * MB], _MM_DTYPE)
            # weights ride the ACT ring in chunks so kw=0 matmuls can
            # start while later chunks stream
            for c in range(4):
                nc.scalar.dma_start(
                    wb_sb[:, c * 1024 : (c + 1) * 1024],
                    wb_d[:, c * 1024 : (c + 1) * 1024],
                )
            nc.scalar.dma_start(wbb_sb[:], wbb_d[:])

            def emit_rep(rep):
                xb = xbp.tile([KB, n_img * WS], _MM_DTYPE, name="xb")
                xb3 = xb.rearrange("p (b w) -> p b w", b=n_img)
                for b in range(n_img):
                    xbase = b * H * W
                    xgs = []
                    for g in range(n_groups):
                        xg = xp.tile([128, GROUP * WS], _MM_DTYPE, name="xg")
                        xg3 = xg.rearrange("p (t w) -> p t w", t=GROUP)
                        t0 = g * GROUP
                        # one DMA per 4-tile group: uniform window AP
                        # reading WS=W+8 consecutive elements per row (the
                        # 8-col spill into the next row feeds only the
                        # discarded pad-j column). The very last tile of
                        # the last image would spill past the x buffer, so
                        # it loads separately with an in-bounds pad.
                        nu = GROUP
                        if b == n_img - 1 and t0 + GROUP == n_tiles:
                            nu = GROUP - 1
                        src = AP(
                            x_d.tensor,
                            xbase + 128 * t0 * W,
                            [[W, 128], [128 * W, nu], [1, WS]],
                        )
                        nc.sync.dma_start(xg3[:, 0:nu, :], src)
                        for t in range(nu, GROUP):
                            s = 128 * (t0 + t)
                            src = AP(
                                x_d.tensor, xbase + s * W, [[W, 128], [1, W]]
                            )
                            nc.sync.dma_start(xg3[:, t, 0:W], src)
                            # pad cols: any in-bounds data (values unused)
                            srcp = AP(
                                x_d.tensor, xbase + s * W,
                                [[W, 128], [1, STRIDE]],
                            )
                            nc.sync.dma_start(xg3[:, t, W:WS], srcp)
                        xgs.append(xg3)
                    # boundary bands: rows 128*(tau+1) .. +7, tau in 0..6;
                    # second K-copy shifted by one conv stride (8 cols)
                    srcb = AP(
                        x_d.tensor,
                        xbase + 128 * W,
                        [[128 * W, n_bnd], [W, 8], [1, WS]],
                    )
                    nc.sync.dma_start(xb3[0 : KB // 2, b, :], srcb)
                    srcb2 = AP(
                        x_d.tensor,
                        xbase + 128 * W + STRIDE,
                        [[128 * W, n_bnd], [W, 8], [1, WS]],
                    )
                    nc.sync.dma_start(xb3[KB // 2 : KB, b, :], srcb2)

                    ps = [
                        [
                            psp.tile([128, NF], f32, name=f"ps_{b}_{g}_{h}",
                                     tag="ps")
                            for h in range(2)
                        ]
                        for g in range(n_groups)
                    ]
                    for kw in range(KW):
                        for h in range(2):
                            lhsT = wb_sb[
                                :, (kw * 2 + h) * 128 : (kw * 2 + h) * 128 + 128
                            ]
                            for g in range(n_groups):
                                rhs = xgs[g][
                                    :, :,
                                    kw : kw + STRIDE * (nWp - 1) + 1 : STRIDE,
                                ]
                                nc.tensor.matmul(
                                    ps[g][h][:],
                                    lhsT,
                                    rhs,
                                    start=(kw == 0),
                                    stop=(kw == KW - 1),
                                )
                    for g in range(n_groups):
                        for h in range(2):
                            ob = op.tile([128, NF], _OUT_DTYPE, name="ob")
                            nc.vector.tensor_copy(ob[:], ps[g][h][:])
                            nc.scalar.dma_start(out_d[b, g, h], ob[:])

                # batched boundary remainder over all images
                psb = psbp.tile([MB, NB], f32, name=f"psb_{rep}", tag="psb")
                for ph in range(8):
                    rhsb = xb3[:, :, ph : ph + STRIDE * (nWp - 1) + 1 : STRIDE]
                    nc.tensor.matmul(
                        psb[:],
                        wbb_sb[:, ph * MB : (ph + 1) * MB],
                        rhsb,
                        start=(ph == 0),
                        stop=(ph == 7),
                    )
                obb = op.tile([MB, NB], _OUT_DTYPE, name="obb")
                nc.vector.tensor_copy(obb[:], psb[:])
                nc.scalar.dma_start(outb_d[:], obb[:])

            if loop > 1:
                with tc.For_i(0, loop):
                    for rep in range(repeat):
                        emit_rep(rep)
            else:
                for rep in range(repeat):
                    emit_rep(rep)
    nc.compile()
    return nc, nH, nW, n_groups


def unscramble(dev_out, dev_outb, nH, nW, n_img):
    """Assemble [n_img, nH*4, nW*4] from the device layouts.

    dev_out  [n_img, n_groups, 2, 128, GROUP*nWp]: main results; split
             patches (i = 16t+15) hold only their kh<8 partial sums.
    dev_outb [16*n_bnd, n_img*nWp]: kh>=8 remainders for split patches.
    """
    n_groups = dev_out.shape[1]
    nWp = ((nW + 3) // 4) * 4
    n_bnd = n_groups * GROUP - 1
    dev = np.asarray(dev_out, np.float32)
    dev = dev.reshape(n_img, n_groups, 2, 8, D0, D1, GROUP, nWp)[..., :nW]
    bnd = np.asarray(dev_outb, np.float32)
    bnd = bnd.reshape(n_bnd, D0, D1, n_img, nWp)[..., :nW]
    out5 = np.empty((n_img, nH, D0, nW, D1), np.float32)
    for g in range(n_groups):
        for t in range(GROUP):
            tau = g * GROUP + t
            for h in range(2):
                for il in range(8):
                    ig = il + 8 * h
                    i = SLOTS * tau + ig
                    if i >= nH:
                        continue
                    # dev[b, g, h, il, d0, d1, t, j] -> out5[b, i, d0, j, d1]
                    v = dev[:, g, h, il, :, :, t, :].transpose(0, 1, 3, 2)
                    if ig == SLOTS - 1:
                        # add the kh>=8 remainder: bnd[tau, d0, d1, b, j]
                        v = v + bnd[tau].transpose(2, 0, 3, 1)
                    out5[:, i] = v
    return out5.reshape(n_img, nH * D0, nW * D1)


def _pjrt_timed_run(nc, in_maps, n_iters=6):
    """Clone of bass2jax.run_bass_via_pjrt's multi-core path, without
    donation, with device-resident inputs, timing each execution.
    Returns (results_per_core, [wall_seconds per iter])."""
    import time

    import jax
    from jax.sharding import Mesh, PartitionSpec
    from jax.experimental.shard_map import shard_map

    from concourse import bass2jax, mybir as _mb
    from concourse.bass2jax import _bass_exec_p, partition_id_tensor

    bass2jax.install_neuronx_cc_hook()
    n_cores = len(in_maps)
    partition_name = nc.partition_id_tensor.name if nc.partition_id_tensor else None

    in_names, out_names, out_avals = [], [], []
    zero_outs = []
    for alloc in nc.m.functions[0].allocations:
        if not isinstance(alloc, _mb.MemoryLocationSet):
            continue
        name = alloc.memorylocations[0].name
        if alloc.kind == "ExternalInput":
            if name != partition_name:
                in_names.append(name)
        elif alloc.kind == "ExternalOutput":
            shape = tuple(alloc.tensor_shape)
            dtype = _mb.dt.np(alloc.dtype)
            out_names.append(name)
            out_avals.append(jax.core.ShapedArray(shape, dtype))
            zero_outs.append(np.zeros(shape, dtype))
    n_params = len(in_names)
    in_names_all = in_names + out_names
    if partition_name is not None:
        in_names_all.append(partition_name)

    def _body(*args):
        operands = list(args)
        if partition_name is not None:
            operands.append(partition_id_tensor())
        outs = _bass_exec_p.bind(
            *operands,
            out_avals=tuple(out_avals),
            in_names=tuple(in_names_all),
            out_names=tuple(out_names),
            lowering_input_output_aliases=(),
            sim_require_finite=True,
            sim_require_nnan=True,
            nc=nc,
        )
        return tuple(outs)

    devices = jax.devices()[:n_cores]
    mesh = Mesh(np.asarray(devices), ("core",))
    in_specs = (PartitionSpec("core"),) * (n_params + len(out_names))
    out_specs = (PartitionSpec("core"),) * len(out_names)
    sharded = jax.jit(
        shard_map(_body, mesh=mesh, in_specs=in_specs, out_specs=out_specs,
                  check_rep=False),
        keep_unused=True,
    )
    concat_in = [
        np.concatenate([np.asarray(in_maps[c][n]) for c in range(n_cores)], axis=0)
        for n in in_names
    ]
    concat_zeros = [
        np.zeros((n_cores * z.shape[0], *z.shape[1:]), z.dtype) for z in zero_outs
    ]
    from jax.sharding import NamedSharding

    dev_in = [
        jax.device_put(a, NamedSharding(mesh, PartitionSpec("core")))
        for a in concat_in + concat_zeros
    ]
    out_arrs = sharded(*dev_in)  # warmup + compile
    jax.block_until_ready(out_arrs)
    times = []
    for _ in range(n_iters):
        t0 = time.perf_counter()
        out_arrs = sharded(*dev_in)
        jax.block_until_ready(out_arrs)
        times.append(time.perf_counter() - t0)
    results = [
        {
            n: np.asarray(out_arrs[i]).reshape(n_cores, *out_avals[i].shape)[c]
            for i, n in enumerate(out_names)
        }
        for c in range(n_cores)
    ]
    return results, times


_CACHE = {}


def _get_nc(n_img, H, W, repeat=1, loop=1):
    key = (n_img, H, W, repeat, loop)
    if key not in _CACHE:
        _CACHE[key] = build_nc(n_img, H, W, repeat, loop)
    return _CACHE[key]


def kernel(x, weight, _timed=False, _repeat=1):
    x = np.asarray(x, np.float32)
    weight = np.asarray(weight, np.float32)
    B, H, W = x.shape
    assert B % N_CORES == 0
    n_img = B // N_CORES
    nc, nH, nW, n_groups = _get_nc(n_img, H, W, _repeat)
    wb = build_wband(weight)
    wbb = build_wband_bnd(weight, H // 128 - 1)
    in_maps = [
        {
            "x": np.ascontiguousarray(x[c * n_img : (c + 1) * n_img]).reshape(-1),
            "wb": wb,
            "wbb": wbb,
        }
        for c in range(N_CORES)
    ]
    if _timed:
        results, times = _pjrt_timed_run(nc, in_maps)
    else:
        results = run_bass_kernel_spmd(
            nc, in_maps, core_ids=list(range(N_CORES))
        ).results
        times = None
    shards = [
        unscramble(results[c]["out"], results[c]["outb"], nH, nW, n_img)
        for c in range(N_CORES)
    ]
    full = np.concatenate(shards, axis=0)
    if _timed:
        return full, times
    return full


# revision 15
# speedup vs baseline: 700.3799x; 1.0143x over previous
"""Trainium2 Bass kernel for strided-conv-as-linear (nn_ConvNd_60851096649851).

Computation (see reference): x [B,1024,1024] f32, weight [16,256] f32.
16x16 windows at stride 8 -> 127x127 patches; per patch y = W @ flat(window)
(16 outputs), reshaped to a 4x4 tile of the [B,508,508] output.

Strategy: data-parallel over batch (4 images per core, 8 cores).

Per image: 8 DISJOINT 128-row window tiles (x is read from HBM exactly
once). Rows live on SBUF partitions (natural layout). For each window
tile, out[(i_l,o), j] = sum_kw Wband_kw[row, (i_l,o)]^T @ x[row, 8j+kw]
where Wband_kw is the banded weight (nonzero at row = 8*i_l + kh): 16
accumulating float32r matmuls, K=128, M=128 (i_l in [0,8) x 16 outputs
per half; two halves cover 16 patch slots/tile). Four tiles are batched
in the moving dim (N = 4*128 = 512) so float32r streams 1 cycle/column.

The 16th patch slot of each tile (i = 16t+15, rows 128t+120..128t+135)
crosses the tile boundary: its kh 0..7 rows live in tile t (handled by
the main h=1 matmul band), its kh 8..15 rows are the first 8 rows of
tile t+1. Those remainders are computed by ONE batched boundary matmul
chain (K=7 bands x 8 rows=56, M=7 bands x 16 outs=112, N=4 images x 128)
from a small re-read [56,1032] boundary tile per image; the host adds
the two partial results for those 7 patch rows per image.

All DMA rides the HWDGE rings (the original baseline put ~70 dma_starts
on the gpsimd SWDGE path whose Q7 descriptor emission at ~1.4us each
dominated the runtime): x loads on nc.sync (SP) as one uniform-window
access pattern per 4-tile group, weights + output stores on nc.scalar
(ACT). Outputs are stored as bf16 (halves store traffic; rel err ~2e-3
<< 2e-2 gate). Host prepares banded weights and unscrambles the device
layout into [B,508,508] f32.
"""

import os
import sys

sys.path.insert(0, "/opt/trn_rl_repo")
os.environ.setdefault("JAX_PLATFORMS", "cpu")

import numpy as np

import concourse.bass as bass  # noqa: F401
import concourse.tile as tile
from concourse import bacc, mybir
from concourse.ap import AP
from concourse.bass_utils import run_bass_kernel_spmd

N_CORES = 8
KH = KW = 16
STRIDE = 8
D0 = D1 = 4  # per-patch output tile
OC = 16  # outputs per patch = D0*D1
SLOTS = 16  # patch slots per 128-row tile (slot 15 is split across tiles)
GROUP = 4  # window tiles batched per matmul (moving dim)

_MM_DTYPE = mybir.dt.float32r
_OUT_DTYPE = mybir.dt.bfloat16


def build_wband(weight):
    """Banded weights: [128, KW*2*128] f32.

    wb[p, kw, h, m] = W[o, kh*16+kw] where p = 8*(i_l+8h)+kh, m = i_l*16+o,
    i_l in [0,8). Slot i_l+8h==15 keeps only its kh<8 rows (p<128); the
    kh>=8 remainder comes from the boundary matmul.
    """
    W4 = np.asarray(weight, np.float32).reshape(OC, KH, KW)
    wb = np.zeros((128, KW, 2, 128), np.float32)
    for h in range(2):
        for il in range(8):
            ig = il + 8 * h
            for kh in range(KH):
                p = 8 * ig + kh
                if p >= 128:
                    continue
                wb[p, :, h, il * OC : (il + 1) * OC] = W4[:, kh, :].T
    return np.ascontiguousarray(wb.reshape(128, KW * 2 * 128))


def build_wband_bnd(weight, n_bnd):
    """Boundary band weights: [16*n_bnd, 8*16*n_bnd] f32.

    The kh>=8 remainder of split patch tau (tau in 0..n_bnd-1), read
    against rows 0..7 of tile tau+1. The boundary contraction only needs
    8*n_bnd=56 rows, so TWO copies of the bands are stacked in K (the
    second reads the x bands shifted by one conv stride) and each of the
    8 phase-matmuls covers two kw values at once:
      rows 8*tau+q       : W[o, 8+q, phase]      (x col 8j+phase)
      rows 56 + 8*tau+q  : W[o, 8+q, phase+8]    (x col 8j+8+phase)
    """
    W4 = np.asarray(weight, np.float32).reshape(OC, KH, KW)
    half = 8 * n_bnd
    wbb = np.zeros((2 * half, 8, OC * n_bnd), np.float32)
    for tau in range(n_bnd):
        for q in range(8):
            wbb[8 * tau + q, :, tau * OC : (tau + 1) * OC] = \
                W4[:, 8 + q, 0:8].T
            wbb[half + 8 * tau + q, :, tau * OC : (tau + 1) * OC] = \
                W4[:, 8 + q, 8:16].T
    return np.ascontiguousarray(wbb.reshape(2 * half, 8 * OC * n_bnd))


def build_nc(n_img, H, W, repeat=1, loop=1):
    """Build the per-core Bass program. Returns compiled nc.

    repeat: python-unrolled copies of the whole per-core computation.
    loop: hardware For_i trip count around those copies (timing only).
    """
    nH = (H - KH) // STRIDE + 1
    nW = (W - KW) // STRIDE + 1
    n_tiles = H // 128  # disjoint tiles
    assert n_tiles % GROUP == 0, (n_tiles, GROUP)
    n_groups = n_tiles // GROUP
    n_bnd = n_tiles - 1  # split patches per image
    KB = 16 * n_bnd  # boundary contraction size (two phase-shifted copies)
    MB = OC * n_bnd  # boundary output partitions
    nWp = nW  # no pad-j: moving AP is [j, t] with t innermost (even)
    NF = GROUP * nWp  # moving free size per main matmul
    NB = n_img * nWp  # moving free size per boundary matmul
    WS = W  # no pad column -> no spill past the row
    assert NB <= 512 and NF <= 512

    nc = bacc.Bacc(
        "TRN2", target_bir_lowering=False, debug=False, num_devices=N_CORES
    )
    f32 = mybir.dt.float32
    x_d = nc.dram_tensor(
        "x", [n_img * H * W], _MM_DTYPE, kind="ExternalInput"
    ).ap()
    wb_d = nc.dram_tensor(
        "wb", [128, KW * 2 * 128], _MM_DTYPE, kind="ExternalInput"
    ).ap()
    wbb_d = nc.dram_tensor(
        "wbb", [KB, 8 * MB], _MM_DTYPE, kind="ExternalInput"
    ).ap()
    out_d = nc.dram_tensor(
        "out", [n_img, n_groups, 2, 128, NF], _OUT_DTYPE, kind="ExternalOutput"
    ).ap()
    outb_d = nc.dram_tensor(
        "outb", [MB, NB], _OUT_DTYPE, kind="ExternalOutput"
    ).ap()

    with tile.TileContext(nc) as tc:
        with (
            tc.tile_pool(name="wbp", bufs=1) as wbp,
            tc.tile_pool(name="xp", bufs=6) as xp,
            tc.tile_pool(name="xbp", bufs=1 if repeat == 1 else 2) as xbp,
            tc.tile_pool(name="psp", bufs=6, space="PSUM") as psp,
            tc.tile_pool(name="psbp", bufs=1 if repeat == 1 else 2,
                         space="PSUM") as psbp,
            tc.tile_pool(name="op", bufs=6) as op,
        ):
            wb_sb = wbp.tile([128, KW * 2 * 128], _MM_DTYPE)
            wbb_sb = wbp.tile([KB, 8 * MB], _MM_DTYPE)
            # weights ride the ACT ring in chunks so kw=0 matmuls can
            # start while later chunks stream
            for c in range(4):
                nc.scalar.dma_start(
                    wb_sb[:, c * 1024 : (c + 1) * 1024],
                    wb_d[:, c * 1024 : (c + 1) * 1024],
                )
            nc.scalar.dma_start(wbb_sb[:], wbb_d[:])

            def emit_rep(rep):
                xb = xbp.tile([KB, n_img * WS], _MM_DTYPE, name="xb")
                xb3 = xb.rearrange("p (b w) -> p b w", b=n_img)
                for b in range(n_img):
                    xbase = b * H * W
                    xgs = []
                    for g in range(n_groups):
                        xg = xp.tile([128, GROUP * WS], _MM_DTYPE, name="xg")
                        xg3 = xg.rearrange("p (t w) -> p t w", t=GROUP)
                        t0 = g * GROUP
                        # one DMA per 4-tile group: uniform window AP
                        src = AP(
                            x_d.tensor,
                            xbase + 128 * t0 * W,
                            [[W, 128], [128 * W, GROUP], [1, WS]],
                        )
                        nc.sync.dma_start(xg3[:, :, :], src)
                        xgs.append(xg3)
                    # boundary bands: rows 128*(tau+1) .. +7, tau in 0..6;
                    # second K-copy shifted by one conv stride (8 cols)
                    srcb = AP(
                        x_d.tensor,
                        xbase + 128 * W,
                        [[128 * W, n_bnd], [W, 8], [1, WS]],
                    )
                    nc.sync.dma_start(xb3[0 : KB // 2, b, :], srcb)
                    srcb2 = AP(
                        x_d.tensor,
                        xbase + 128 * W + STRIDE,
                        [[128 * W, n_bnd], [W, 8], [1, WS]],
                    )
                    nc.sync.dma_start(xb3[KB // 2 : KB, b, :], srcb2)

                    ps = [
                        [
                            psp.tile([128, NF], f32, name=f"ps_{b}_{g}_{h}",
                                     tag="ps")
                            for h in range(2)
                        ]
                        for g in range(n_groups)
                    ]
                    for kw in range(KW):
                        for h in range(2):
                            lhsT = wb_sb[
                                :, (kw * 2 + h) * 128 : (kw * 2 + h) * 128 + 128
                            ]
                            for g in range(n_groups):
                                rhs = xgs[g][
                                    :, :,
                                    kw : kw + STRIDE * (nWp - 1) + 1 : STRIDE,
                                ].transpose([0, 2, 1])
                                nc.tensor.matmul(
                                    ps[g][h][:],
                                    lhsT,
                                    rhs,
                                    start=(kw == 0),
                                    stop=(kw == KW - 1),
                                )
                    for g in range(n_groups):
                        for h in range(2):
                            ob = op.tile([128, NF], _OUT_DTYPE, name="ob")
                            nc.vector.tensor_copy(ob[:], ps[g][h][:])
                            nc.scalar.dma_start(out_d[b, g, h], ob[:])

                # batched boundary remainder over all images
                psb = psbp.tile([MB, NB], f32, name=f"psb_{rep}", tag="psb")
                for ph in range(8):
                    rhsb = xb3[
                        :, :, ph : ph + STRIDE * (nWp - 1) + 1 : STRIDE
                    ].transpose([0, 2, 1])
                    nc.tensor.matmul(
                        psb[:],
                        wbb_sb[:, ph * MB : (ph + 1) * MB],
                        rhsb,
                        start=(ph == 0),
                        stop=(ph == 7),
                    )
                obb = op.tile([MB, NB], _OUT_DTYPE, name="obb")
                nc.vector.tensor_copy(obb[:], psb[:])
                nc.scalar.dma_start(outb_d[:], obb[:])

            if loop > 1:
                with tc.For_i(0, loop):
                    for rep in range(repeat):
                        emit_rep(rep)
            else:
                for rep in range(repeat):
                    emit_rep(rep)
    nc.compile()
    return nc, nH, nW, n_groups


def unscramble(dev_out, dev_outb, nH, nW, n_img):
    """Assemble [n_img, nH*4, nW*4] from the device layouts.

    dev_out  [n_img, n_groups, 2, 128, GROUP*nWp]: main results; split
             patches (i = 16t+15) hold only their kh<8 partial sums.
    dev_outb [16*n_bnd, n_img*nWp]: kh>=8 remainders for split patches.
    """
    n_groups = dev_out.shape[1]
    nWp = nW
    n_bnd = n_groups * GROUP - 1
    dev = np.asarray(dev_out, np.float32)
    dev = dev.reshape(n_img, n_groups, 2, 8, D0, D1, nWp, GROUP)
    bnd = np.asarray(dev_outb, np.float32)
    bnd = bnd.reshape(n_bnd, D0, D1, nWp, n_img)
    out5 = np.empty((n_img, nH, D0, nW, D1), np.float32)
    for g in range(n_groups):
        for t in range(GROUP):
            tau = g * GROUP + t
            for h in range(2):
                for il in range(8):
                    ig = il + 8 * h
                    i = SLOTS * tau + ig
                    if i >= nH:
                        continue
                    # dev[b, g, h, il, d0, d1, j, t] -> out5[b, i, d0, j, d1]
                    v = dev[:, g, h, il, :, :, :, t].transpose(0, 1, 3, 2)
                    if ig == SLOTS - 1:
                        # add the kh>=8 remainder: bnd[tau, d0, d1, j, b]
                        v = v + bnd[tau].transpose(3, 0, 2, 1)
                    out5[:, i] = v
    return out5.reshape(n_img, nH * D0, nW * D1)


def _pjrt_timed_run(nc, in_maps, n_iters=6):
    """Clone of bass2jax.run_bass_via_pjrt's multi-core path, without
    donation, with device-resident inputs, timing each execution.
    Returns (results_per_core, [wall_seconds per iter])."""
    import time

    import jax
    from jax.sharding import Mesh, PartitionSpec
    from jax.experimental.shard_map import shard_map

    from concourse import bass2jax, mybir as _mb
    from concourse.bass2jax import _bass_exec_p, partition_id_tensor

    bass2jax.install_neuronx_cc_hook()
    n_cores = len(in_maps)
    partition_name = nc.partition_id_tensor.name if nc.partition_id_tensor else None

    in_names, out_names, out_avals = [], [], []
    zero_outs = []
    for alloc in nc.m.functions[0].allocations:
        if not isinstance(alloc, _mb.MemoryLocationSet):
            continue
        name = alloc.memorylocations[0].name
        if alloc.kind == "ExternalInput":
            if name != partition_name:
                in_names.append(name)
        elif alloc.kind == "ExternalOutput":
            shape = tuple(alloc.tensor_shape)
            dtype = _mb.dt.np(alloc.dtype)
            out_names.append(name)
            out_avals.append(jax.core.ShapedArray(shape, dtype))
            zero_outs.append(np.zeros(shape, dtype))
    n_params = len(in_names)
    in_names_all = in_names + out_names
    if partition_name is not None:
        in_names_all.append(partition_name)

    def _body(*args):
        operands = list(args)
        if partition_name is not None:
            operands.append(partition_id_tensor())
        outs = _bass_exec_p.bind(
            *operands,
            out_avals=tuple(out_avals),
            in_names=tuple(in_names_all),
            out_names=tuple(out_names),
            lowering_input_output_aliases=(),
            sim_require_finite=True,
            sim_require_nnan=True,
            nc=nc,
        )
        return tuple(outs)

    devices = jax.devices()[:n_cores]
    mesh = Mesh(np.asarray(devices), ("core",))
    in_specs = (PartitionSpec("core"),) * (n_params + len(out_names))
    out_specs = (PartitionSpec("core"),) * len(out_names)
    sharded = jax.jit(
        shard_map(_body, mesh=mesh, in_specs=in_specs, out_specs=out_specs,
                  check_rep=False),
        keep_unused=True,
    )
    concat_in = [
        np.concatenate([np.asarray(in_maps[c][n]) for c in range(n_cores)], axis=0)
        for n in in_names
    ]
    concat_zeros = [
        np.zeros((n_cores * z.shape[0], *z.shape[1:]), z.dtype) for z in zero_outs
    ]
    from jax.sharding import NamedSharding

    dev_in = [
        jax.device_put(a, NamedSharding(mesh, PartitionSpec("core")))
        for a in concat_in + concat_zeros
    ]
    out_arrs = sharded(*dev_in)  # warmup + compile
    jax.block_until_ready(out_arrs)
    times = []
    for _ in range(n_iters):
        t0 = time.perf_counter()
        out_arrs = sharded(*dev_in)
        jax.block_until_ready(out_arrs)
        times.append(time.perf_counter() - t0)
    results = [
        {
            n: np.asarray(out_arrs[i]).reshape(n_cores, *out_avals[i].shape)[c]
            for i, n in enumerate(out_names)
        }
        for c in range(n_cores)
    ]
    return results, times


_CACHE = {}


def _get_nc(n_img, H, W, repeat=1, loop=1):
    key = (n_img, H, W, repeat, loop)
    if key not in _CACHE:
        _CACHE[key] = build_nc(n_img, H, W, repeat, loop)
    return _CACHE[key]


def kernel(x, weight, _timed=False, _repeat=1):
    x = np.asarray(x, np.float32)
    weight = np.asarray(weight, np.float32)
    B, H, W = x.shape
    assert B % N_CORES == 0
    n_img = B // N_CORES
    nc, nH, nW, n_groups = _get_nc(n_img, H, W, _repeat)
    wb = build_wband(weight)
    wbb = build_wband_bnd(weight, H // 128 - 1)
    in_maps = [
        {
            "x": np.ascontiguousarray(x[c * n_img : (c + 1) * n_img]).reshape(-1),
            "wb": wb,
            "wbb": wbb,
        }
        for c in range(N_CORES)
    ]
    if _timed:
        results, times = _pjrt_timed_run(nc, in_maps)
    else:
        results = run_bass_kernel_spmd(
            nc, in_maps, core_ids=list(range(N_CORES))
        ).results
        times = None
    shards = [
        unscramble(results[c]["out"], results[c]["outb"], nH, nW, n_img)
        for c in range(N_CORES)
    ]
    full = np.concatenate(shards, axis=0)
    if _timed:
        return full, times
    return full


# revision 16
# speedup vs baseline: 700.4510x; 1.0001x over previous
"""Trainium2 Bass kernel for strided-conv-as-linear (nn_ConvNd_60851096649851).

Computation (see reference): x [B,1024,1024] f32, weight [16,256] f32.
16x16 windows at stride 8 -> 127x127 patches; per patch y = W @ flat(window)
(16 outputs), reshaped to a 4x4 tile of the [B,508,508] output.

Strategy: data-parallel over batch (4 images per core, 8 cores).

Per image: 8 DISJOINT 128-row window tiles (x is read from HBM exactly
once). Rows live on SBUF partitions (natural layout). For each window
tile, out[(i_l,o), j] = sum_kw Wband_kw[row, (i_l,o)]^T @ x[row, 8j+kw]
where Wband_kw is the banded weight (nonzero at row = 8*i_l + kh): 16
accumulating float32r matmuls, K=128, M=128 (i_l in [0,8) x 16 outputs
per half; two halves cover 16 patch slots/tile). Four tiles are batched
in the moving dim (N = 4*128 = 512) so float32r streams 1 cycle/column.

The moving AP is [j, t] with t innermost (size 4, even, satisfying the
fp32r even-moving constraint) so no pad-j column is needed: N = 4*127 =
508 and the loads read exactly W columns per row (no spill handling).

The 16th patch slot of each tile (i = 16t+15, rows 128t+120..128t+135)
crosses the tile boundary: its kh 0..7 rows live in tile t (handled by
the main h=1 matmul band), its kh 8..15 rows are the first 8 rows of
tile t+1. Those remainders are computed by ONE batched boundary matmul
chain per rep: two column-shifted copies of the 7 boundary bands are
stacked in K (K=112, M=112) so 8 phase-matmuls (N=4 images x 127) cover
all 16 kw; the host adds the two partials for those 7 patch rows.

All DMA rides the HWDGE rings (the original baseline put ~70 dma_starts
on the gpsimd SWDGE path whose Q7 descriptor emission at ~1.4us each
dominated the runtime): x loads on nc.sync (SP) as one uniform-window
access pattern per 4-tile group, weights + output stores on nc.scalar
(ACT). Outputs are stored as bf16 (halves store traffic; rel err ~2e-3
<< 2e-2 gate). Host prepares banded weights and unscrambles the device
layout into [B,508,508] f32.
"""

import os
import sys

sys.path.insert(0, "/opt/trn_rl_repo")
os.environ.setdefault("JAX_PLATFORMS", "cpu")

import numpy as np

import concourse.bass as bass  # noqa: F401
import concourse.tile as tile
from concourse import bacc, mybir
from concourse.ap import AP
from concourse.bass_utils import run_bass_kernel_spmd

N_CORES = 8
KH = KW = 16
STRIDE = 8
D0 = D1 = 4  # per-patch output tile
OC = 16  # outputs per patch = D0*D1
SLOTS = 16  # patch slots per 128-row tile (slot 15 is split across tiles)
GROUP = 4  # window tiles batched per matmul (moving dim)

_MM_DTYPE = mybir.dt.float32r
_OUT_DTYPE = mybir.dt.bfloat16


def build_wband(weight):
    """Banded weights: [128, KW*2*128] f32.

    wb[p, kw, h, m] = W[o, kh*16+kw] where p = 8*(i_l+8h)+kh, m = i_l*16+o,
    i_l in [0,8). Slot i_l+8h==15 keeps only its kh<8 rows (p<128); the
    kh>=8 remainder comes from the boundary matmul.
    """
    W4 = np.asarray(weight, np.float32).reshape(OC, KH, KW)
    wb = np.zeros((128, KW, 2, 128), np.float32)
    for h in range(2):
        for il in range(8):
            ig = il + 8 * h
            for kh in range(KH):
                p = 8 * ig + kh
                if p >= 128:
                    continue
                wb[p, :, h, il * OC : (il + 1) * OC] = W4[:, kh, :].T
    return np.ascontiguousarray(wb.reshape(128, KW * 2 * 128))


def build_wband_bnd(weight, n_bnd):
    """Boundary band weights: [16*n_bnd, 8*16*n_bnd] f32.

    The kh>=8 remainder of split patch tau (tau in 0..n_bnd-1), read
    against rows 0..7 of tile tau+1. The boundary contraction only needs
    8*n_bnd=56 rows, so TWO copies of the bands are stacked in K (the
    second reads the x bands shifted by one conv stride) and each of the
    8 phase-matmuls covers two kw values at once:
      rows 8*tau+q       : W[o, 8+q, phase]      (x col 8j+phase)
      rows 56 + 8*tau+q  : W[o, 8+q, phase+8]    (x col 8j+8+phase)
    """
    W4 = np.asarray(weight, np.float32).reshape(OC, KH, KW)
    half = 8 * n_bnd
    wbb = np.zeros((2 * half, 8, OC * n_bnd), np.float32)
    for tau in range(n_bnd):
        for q in range(8):
            wbb[8 * tau + q, :, tau * OC : (tau + 1) * OC] = \
                W4[:, 8 + q, 0:8].T
            wbb[half + 8 * tau + q, :, tau * OC : (tau + 1) * OC] = \
                W4[:, 8 + q, 8:16].T
    return np.ascontiguousarray(wbb.reshape(2 * half, 8 * OC * n_bnd))


def build_nc(n_img, H, W, repeat=1, loop=1):
    """Build the per-core Bass program. Returns compiled nc.

    repeat: python-unrolled copies of the whole per-core computation.
    loop: hardware For_i trip count around those copies (timing only).
    """
    nH = (H - KH) // STRIDE + 1
    nW = (W - KW) // STRIDE + 1
    n_tiles = H // 128  # disjoint tiles
    assert n_tiles % GROUP == 0, (n_tiles, GROUP)
    n_groups = n_tiles // GROUP
    n_bnd = n_tiles - 1  # split patches per image
    KB = 16 * n_bnd  # boundary contraction size (two phase-shifted copies)
    MB = OC * n_bnd  # boundary output partitions
    nWp = nW  # no pad-j: moving AP is [j, t] with t innermost (even)
    NF = GROUP * nWp  # moving free size per main matmul
    NB = n_img * nWp  # moving free size per boundary matmul
    WS = W  # no pad column -> no spill past the row
    assert NB <= 512 and NF <= 512

    nc = bacc.Bacc(
        "TRN2", target_bir_lowering=False, debug=False, num_devices=N_CORES
    )
    f32 = mybir.dt.float32
    x_d = nc.dram_tensor(
        "x", [n_img * H * W], _MM_DTYPE, kind="ExternalInput"
    ).ap()
    wb_d = nc.dram_tensor(
        "wb", [128, KW * 2 * 128], _MM_DTYPE, kind="ExternalInput"
    ).ap()
    wbb_d = nc.dram_tensor(
        "wbb", [KB, 8 * MB], _MM_DTYPE, kind="ExternalInput"
    ).ap()
    out_d = nc.dram_tensor(
        "out", [n_img, n_groups, 2, 128, NF], _OUT_DTYPE, kind="ExternalOutput"
    ).ap()
    outb_d = nc.dram_tensor(
        "outb", [MB, NB], _OUT_DTYPE, kind="ExternalOutput"
    ).ap()

    with tile.TileContext(nc) as tc:
        with (
            tc.tile_pool(name="wbp", bufs=1) as wbp,
            tc.tile_pool(name="xp", bufs=7) as xp,
            tc.tile_pool(name="xbp", bufs=1 if repeat == 1 else 2) as xbp,
            tc.tile_pool(name="psp", bufs=6, space="PSUM") as psp,
            tc.tile_pool(name="psbp", bufs=1 if repeat == 1 else 2,
                         space="PSUM") as psbp,
            tc.tile_pool(name="op", bufs=8) as op,
        ):
            wb_sb = wbp.tile([128, KW * 2 * 128], _MM_DTYPE)
            wbb_sb = wbp.tile([KB, 8 * MB], _MM_DTYPE)
            # weights ride the ACT ring in chunks so kw=0 matmuls can
            # start while later chunks stream
            for c in range(4):
                nc.scalar.dma_start(
                    wb_sb[:, c * 1024 : (c + 1) * 1024],
                    wb_d[:, c * 1024 : (c + 1) * 1024],
                )
            nc.scalar.dma_start(wbb_sb[:], wbb_d[:])

            def emit_rep(rep):
                xb = xbp.tile([KB, n_img * WS], _MM_DTYPE, name="xb")
                xb3 = xb.rearrange("p (b w) -> p b w", b=n_img)
                for b in range(n_img):
                    xbase = b * H * W
                    xgs = []
                    for g in range(n_groups):
                        xg = xp.tile([128, GROUP * WS], _MM_DTYPE, name="xg")
                        xg3 = xg.rearrange("p (t w) -> p t w", t=GROUP)
                        t0 = g * GROUP
                        # one DMA per 4-tile group: uniform window AP
                        src = AP(
                            x_d.tensor,
                            xbase + 128 * t0 * W,
                            [[W, 128], [128 * W, GROUP], [1, WS]],
                        )
                        nc.sync.dma_start(xg3[:, :, :], src)
                        xgs.append(xg3)
                    # boundary bands: rows 128*(tau+1) .. +7, tau in 0..6;
                    # second K-copy shifted by one conv stride (8 cols)
                    srcb = AP(
                        x_d.tensor,
                        xbase + 128 * W,
                        [[128 * W, n_bnd], [W, 8], [1, WS]],
                    )
                    nc.sync.dma_start(xb3[0 : KB // 2, b, :], srcb)
                    srcb2 = AP(
                        x_d.tensor,
                        xbase + 128 * W + STRIDE,
                        [[128 * W, n_bnd], [W, 8], [1, WS]],
                    )
                    nc.sync.dma_start(xb3[KB // 2 : KB, b, :], srcb2)

                    ps = [
                        [
                            psp.tile([128, NF], f32, name=f"ps_{b}_{g}_{h}",
                                     tag="ps")
                            for h in range(2)
                        ]
                        for g in range(n_groups)
                    ]
                    for kw in range(KW):
                        for h in range(2):
                            lhsT = wb_sb[
                                :, (kw * 2 + h) * 128 : (kw * 2 + h) * 128 + 128
                            ]
                            for g in range(n_groups):
                                rhs = xgs[g][
                                    :, :,
                                    kw : kw + STRIDE * (nWp - 1) + 1 : STRIDE,
                                ].transpose([0, 2, 1])
                                nc.tensor.matmul(
                                    ps[g][h][:],
                                    lhsT,
                                    rhs,
                                    start=(kw == 0),
                                    stop=(kw == KW - 1),
                                )
                    for g in range(n_groups):
                        for h in range(2):
                            ob = op.tile([128, NF], _OUT_DTYPE, name="ob")
                            nc.vector.tensor_copy(ob[:], ps[g][h][:])
                            nc.scalar.dma_start(out_d[b, g, h], ob[:])

                # batched boundary remainder over all images
                psb = psbp.tile([MB, NB], f32, name=f"psb_{rep}", tag="psb")
                for ph in range(8):
                    rhsb = xb3[
                        :, :, ph : ph + STRIDE * (nWp - 1) + 1 : STRIDE
                    ].transpose([0, 2, 1])
                    nc.tensor.matmul(
                        psb[:],
                        wbb_sb[:, ph * MB : (ph + 1) * MB],
                        rhsb,
                        start=(ph == 0),
                        stop=(ph == 7),
                    )
                obb = op.tile([MB, NB], _OUT_DTYPE, name="obb")
                nc.vector.tensor_copy(obb[:], psb[:])
                nc.scalar.dma_start(outb_d[:], obb[:])

            if loop > 1:
                with tc.For_i(0, loop):
                    for rep in range(repeat):
                        emit_rep(rep)
            else:
                for rep in range(repeat):
                    emit_rep(rep)
    nc.compile()
    return nc, nH, nW, n_groups


def unscramble(dev_out, dev_outb, nH, nW, n_img):
    """Assemble [n_img, nH*4, nW*4] from the device layouts.

    dev_out  [n_img, n_groups, 2, 128, GROUP*nWp]: main results; split
             patches (i = 16t+15) hold only their kh<8 partial sums.
    dev_outb [16*n_bnd, n_img*nWp]: kh>=8 remainders for split patches.
    """
    n_groups = dev_out.shape[1]
    nWp = nW
    n_bnd = n_groups * GROUP - 1
    dev = np.asarray(dev_out, np.float32)
    dev = dev.reshape(n_img, n_groups, 2, 8, D0, D1, nWp, GROUP)
    bnd = np.asarray(dev_outb, np.float32)
    bnd = bnd.reshape(n_bnd, D0, D1, nWp, n_img)
    out5 = np.empty((n_img, nH, D0, nW, D1), np.float32)
    for g in range(n_groups):
        for t in range(GROUP):
            tau = g * GROUP + t
            for h in range(2):
                for il in range(8):
                    ig = il + 8 * h
                    i = SLOTS * tau + ig
                    if i >= nH:
                        continue
                    # dev[b, g, h, il, d0, d1, j, t] -> out5[b, i, d0, j, d1]
                    v = dev[:, g, h, il, :, :, :, t].transpose(0, 1, 3, 2)
                    if ig == SLOTS - 1:
                        # add the kh>=8 remainder: bnd[tau, d0, d1, j, b]
                        v = v + bnd[tau].transpose(3, 0, 2, 1)
                    out5[:, i] = v
    return out5.reshape(n_img, nH * D0, nW * D1)


def _pjrt_timed_run(nc, in_maps, n_iters=6):
    """Clone of bass2jax.run_bass_via_pjrt's multi-core path, without
    donation, with device-resident inputs, timing each execution.
    Returns (results_per_core, [wall_seconds per iter])."""
    import time

    import jax
    from jax.sharding import Mesh, PartitionSpec
    from jax.experimental.shard_map import shard_map

    from concourse import bass2jax, mybir as _mb
    from concourse.bass2jax import _bass_exec_p, partition_id_tensor

    bass2jax.install_neuronx_cc_hook()
    n_cores = len(in_maps)
    partition_name = nc.partition_id_tensor.name if nc.partition_id_tensor else None

    in_names, out_names, out_avals = [], [], []
    zero_outs = []
    for alloc in nc.m.functions[0].allocations:
        if not isinstance(alloc, _mb.MemoryLocationSet):
            continue
        name = alloc.memorylocations[0].name
        if alloc.kind == "ExternalInput":
            if name != partition_name:
                in_names.append(name)
        elif alloc.kind == "ExternalOutput":
            shape = tuple(alloc.tensor_shape)
            dtype = _mb.dt.np(alloc.dtype)
            out_names.append(name)
            out_avals.append(jax.core.ShapedArray(shape, dtype))
            zero_outs.append(np.zeros(shape, dtype))
    n_params = len(in_names)
    in_names_all = in_names + out_names
    if partition_name is not None:
        in_names_all.append(partition_name)

    def _body(*args):
        operands = list(args)
        if partition_name is not None:
            operands.append(partition_id_tensor())
        outs = _bass_exec_p.bind(
            *operands,
            out_avals=tuple(out_avals),
            in_names=tuple(in_names_all),
            out_names=tuple(out_names),
            lowering_input_output_aliases=(),
            sim_require_finite=True,
            sim_require_nnan=True,
            nc=nc,
        )
        return tuple(outs)

    devices = jax.devices()[:n_cores]
    mesh = Mesh(np.asarray(devices), ("core",))
    in_specs = (PartitionSpec("core"),) * (n_params + len(out_names))
    out_specs = (PartitionSpec("core"),) * len(out_names)
    sharded = jax.jit(
        shard_map(_body, mesh=mesh, in_specs=in_specs, out_specs=out_specs,
                  check_rep=False),
        keep_unused=True,
    )
    concat_in = [
        np.concatenate([np.asarray(in_maps[c][n]) for c in range(n_cores)], axis=0)
        for n in in_names
    ]
    concat_zeros = [
        np.zeros((n_cores * z.shape[0], *z.shape[1:]), z.dtype) for z in zero_outs
    ]
    from jax.sharding import NamedSharding

    dev_in = [
        jax.device_put(a, NamedSharding(mesh, PartitionSpec("core")))
        for a in concat_in + concat_zeros
    ]
    out_arrs = sharded(*dev_in)  # warmup + compile
    jax.block_until_ready(out_arrs)
    times = []
    for _ in range(n_iters):
        t0 = time.perf_counter()
        out_arrs = sharded(*dev_in)
        jax.block_until_ready(out_arrs)
        times.append(time.perf_counter() - t0)
    results = [
        {
            n: np.asarray(out_arrs[i]).reshape(n_cores, *out_avals[i].shape)[c]
            for i, n in enumerate(out_names)
        }
        for c in range(n_cores)
    ]
    return results, times


_CACHE = {}


def _get_nc(n_img, H, W, repeat=1, loop=1):
    key = (n_img, H, W, repeat, loop)
    if key not in _CACHE:
        _CACHE[key] = build_nc(n_img, H, W, repeat, loop)
    return _CACHE[key]


def kernel(x, weight, _timed=False, _repeat=1):
    x = np.asarray(x, np.float32)
    weight = np.asarray(weight, np.float32)
    B, H, W = x.shape
    assert B % N_CORES == 0
    n_img = B // N_CORES
    nc, nH, nW, n_groups = _get_nc(n_img, H, W, _repeat)
    wb = build_wband(weight)
    wbb = build_wband_bnd(weight, H // 128 - 1)
    in_maps = [
        {
            "x": np.ascontiguousarray(x[c * n_img : (c + 1) * n_img]).reshape(-1),
            "wb": wb,
            "wbb": wbb,
        }
        for c in range(N_CORES)
    ]
    if _timed:
        results, times = _pjrt_timed_run(nc, in_maps)
    else:
        results = run_bass_kernel_spmd(
            nc, in_maps, core_ids=list(range(N_CORES))
        ).results
        times = None
    shards = [
        unscramble(results[c]["out"], results[c]["outb"], nH, nW, n_img)
        for c in range(N_CORES)
    ]
    full = np.concatenate(shards, axis=0)
    if _timed:
        return full, times
    return full


# revision 18
# speedup vs baseline: 711.0533x; 1.0151x over previous
"""Trainium2 Bass kernel for strided-conv-as-linear (nn_ConvNd_60851096649851).

Computation (see reference): x [B,1024,1024] f32, weight [16,256] f32.
16x16 windows at stride 8 -> 127x127 patches; per patch y = W @ flat(window)
(16 outputs), reshaped to a 4x4 tile of the [B,508,508] output.

Strategy: data-parallel over batch (4 images per core, 8 cores).

Per image: 8 DISJOINT 128-row window tiles (x is read from HBM exactly
once). Rows live on SBUF partitions (natural layout). For each window
tile, out[(i_l,o), j] = sum_kw Wband_kw[row, (i_l,o)]^T @ x[row, 8j+kw]
where Wband_kw is the banded weight (nonzero at row = 8*i_l + kh): 16
accumulating float32r matmuls, K=128, M=128 (i_l in [0,8) x 16 outputs
per half; two halves cover 16 patch slots/tile). Four tiles are batched
in the moving dim (N = 4*128 = 512) so float32r streams 1 cycle/column.

The moving AP is [j, t] with t innermost (size 4, even, satisfying the
fp32r even-moving constraint) so no pad-j column is needed: N = 4*127 =
508 and the loads read exactly W columns per row (no spill handling).

The 16th patch slot of each tile (i = 16t+15, rows 128t+120..128t+135)
crosses the tile boundary: its kh 0..7 rows live in tile t (handled by
the main h=1 matmul band), its kh 8..15 rows are the first 8 rows of
tile t+1. Those remainders are computed by ONE batched boundary matmul
chain per rep: two column-shifted copies of the 7 boundary bands are
stacked in K (K=112, M=112) so 8 phase-matmuls (N=4 images x 127) cover
all 16 kw; the host adds the two partials for those 7 patch rows.

All DMA rides the HWDGE rings (the original baseline put ~70 dma_starts
on the gpsimd SWDGE path whose Q7 descriptor emission at ~1.4us each
dominated the runtime): x loads on nc.sync (SP) as one uniform-window
access pattern per 4-tile group, weights + output stores on nc.scalar
(ACT). Outputs are stored as bf16 (halves store traffic; rel err ~2e-3
<< 2e-2 gate). Host prepares banded weights and unscrambles the device
layout into [B,508,508] f32.
"""

import os
import sys

sys.path.insert(0, "/opt/trn_rl_repo")
os.environ.setdefault("JAX_PLATFORMS", "cpu")

import numpy as np

import concourse.bass as bass  # noqa: F401
import concourse.tile as tile
from concourse import bacc, mybir
from concourse.ap import AP
from concourse.bass_utils import run_bass_kernel_spmd

N_CORES = 8
KH = KW = 16
STRIDE = 8
D0 = D1 = 4  # per-patch output tile
OC = 16  # outputs per patch = D0*D1
SLOTS = 16  # patch slots per 128-row tile (slot 15 is split across tiles)
GROUP = 4  # window tiles batched per matmul (moving dim)

_MM_DTYPE = mybir.dt.float32r
_OUT_DTYPE = mybir.dt.bfloat16


def build_wband(weight):
    """Banded weights: [128, KW*2*128] f32.

    wb[p, kw, h, m] = W[o, kh*16+kw] where p = 8*(i_l+8h)+kh, m = i_l*16+o,
    i_l in [0,8). Slot i_l+8h==15 keeps only its kh<8 rows (p<128); the
    kh>=8 remainder comes from the boundary matmul.
    """
    W4 = np.asarray(weight, np.float32).reshape(OC, KH, KW)
    wb = np.zeros((128, KW, 2, 128), np.float32)
    for h in range(2):
        for il in range(8):
            ig = il + 8 * h
            for kh in range(KH):
                p = 8 * ig + kh
                if p >= 128:
                    continue
                wb[p, :, h, il * OC : (il + 1) * OC] = W4[:, kh, :].T
    return np.ascontiguousarray(wb.reshape(128, KW * 2 * 128))


def build_wband_bnd(weight, n_bnd):
    """Boundary band weights: [16*n_bnd, 8*16*n_bnd] f32.

    The kh>=8 remainder of split patch tau (tau in 0..n_bnd-1), read
    against rows 0..7 of tile tau+1. The boundary contraction only needs
    8*n_bnd=56 rows, so TWO copies of the bands are stacked in K (the
    second reads the x bands shifted by one conv stride) and each of the
    8 phase-matmuls covers two kw values at once:
      rows 8*tau+q       : W[o, 8+q, phase]      (x col 8j+phase)
      rows 56 + 8*tau+q  : W[o, 8+q, phase+8]    (x col 8j+8+phase)
    """
    W4 = np.asarray(weight, np.float32).reshape(OC, KH, KW)
    half = 8 * n_bnd
    wbb = np.zeros((2 * half, 8, OC * n_bnd), np.float32)
    for tau in range(n_bnd):
        for q in range(8):
            wbb[8 * tau + q, :, tau * OC : (tau + 1) * OC] = \
                W4[:, 8 + q, 0:8].T
            wbb[half + 8 * tau + q, :, tau * OC : (tau + 1) * OC] = \
                W4[:, 8 + q, 8:16].T
    return np.ascontiguousarray(wbb.reshape(2 * half, 8 * OC * n_bnd))


def build_nc(n_img, H, W, repeat=1, loop=1):
    """Build the per-core Bass program. Returns compiled nc.

    repeat: python-unrolled copies of the whole per-core computation.
    loop: hardware For_i trip count around those copies (timing only).
    """
    nH = (H - KH) // STRIDE + 1
    nW = (W - KW) // STRIDE + 1
    n_tiles = H // 128  # disjoint tiles
    assert n_tiles % GROUP == 0, (n_tiles, GROUP)
    n_groups = n_tiles // GROUP
    n_bnd = n_tiles - 1  # split patches per image
    KB = 16 * n_bnd  # boundary contraction size (two phase-shifted copies)
    MB = OC * n_bnd  # boundary output partitions
    nWp = nW  # no pad-j: moving AP is [j, t] with t innermost (even)
    NF = GROUP * nWp  # moving free size per main matmul
    NB = n_img * nWp  # moving free size per boundary matmul
    WS = W  # no pad column -> no spill past the row
    assert NB <= 512 and NF <= 512

    nc = bacc.Bacc(
        "TRN2", target_bir_lowering=False, debug=False, num_devices=N_CORES
    )
    f32 = mybir.dt.float32
    x_d = nc.dram_tensor(
        "x", [n_img * H * W], _MM_DTYPE, kind="ExternalInput"
    ).ap()
    wb_d = nc.dram_tensor(
        "wb", [128, KW * 2 * 128], _MM_DTYPE, kind="ExternalInput"
    ).ap()
    wbb_d = nc.dram_tensor(
        "wbb", [KB, 8 * MB], _MM_DTYPE, kind="ExternalInput"
    ).ap()
    out_d = nc.dram_tensor(
        "out", [n_img, n_groups, 2, 128, NF], _OUT_DTYPE, kind="ExternalOutput"
    ).ap()
    outb_d = nc.dram_tensor(
        "outb", [MB, NB], _OUT_DTYPE, kind="ExternalOutput"
    ).ap()

    with tile.TileContext(nc) as tc:
        with (
            tc.tile_pool(name="wbp", bufs=1) as wbp,
            tc.tile_pool(name="xp", bufs=7) as xp,
            tc.tile_pool(name="xbp", bufs=1 if repeat == 1 else 2) as xbp,
            tc.tile_pool(name="psp", bufs=7, space="PSUM") as psp,
            tc.tile_pool(name="psbp", bufs=1, space="PSUM") as psbp,
            tc.tile_pool(name="op", bufs=8) as op,
        ):
            wb_sb = wbp.tile([128, KW * 2 * 128], _MM_DTYPE)
            wbb_sb = wbp.tile([KB, 8 * MB], _MM_DTYPE)
            # weights ride the ACT ring in chunks so kw=0 matmuls can
            # start while later chunks stream
            for c in range(4):
                nc.scalar.dma_start(
                    wb_sb[:, c * 1024 : (c + 1) * 1024],
                    wb_d[:, c * 1024 : (c + 1) * 1024],
                )
            nc.scalar.dma_start(wbb_sb[:], wbb_d[:])

            def emit_rep(rep):
                xb = xbp.tile([KB, n_img * WS], _MM_DTYPE, name="xb")
                xb3 = xb.rearrange("p (b w) -> p b w", b=n_img)
                for b in range(n_img):
                    xbase = b * H * W
                    xgs = []
                    for g in range(n_groups):
                        xg = xp.tile([128, GROUP * WS], _MM_DTYPE, name="xg")
                        xg3 = xg.rearrange("p (t w) -> p t w", t=GROUP)
                        t0 = g * GROUP
                        # one DMA per 4-tile group: uniform window AP
                        src = AP(
                            x_d.tensor,
                            xbase + 128 * t0 * W,
                            [[W, 128], [128 * W, GROUP], [1, WS]],
                        )
                        nc.sync.dma_start(xg3[:, :, :], src)
                        xgs.append(xg3)
                    # boundary bands: rows 128*(tau+1) .. +7, tau in 0..6;
                    # second K-copy shifted by one conv stride (8 cols)
                    srcb = AP(
                        x_d.tensor,
                        xbase + 128 * W,
                        [[128 * W, n_bnd], [W, 8], [1, WS]],
                    )
                    nc.scalar.dma_start(xb3[0 : KB // 2, b, :], srcb)
                    srcb2 = AP(
                        x_d.tensor,
                        xbase + 128 * W + STRIDE,
                        [[128 * W, n_bnd], [W, 8], [1, WS]],
                    )
                    nc.scalar.dma_start(xb3[KB // 2 : KB, b, :], srcb2)

                    ps = [
                        [
                            psp.tile([128, NF], f32, name=f"ps_{b}_{g}_{h}",
                                     tag="ps")
                            for h in range(2)
                        ]
                        for g in range(n_groups)
                    ]
                    for kw in range(KW):
                        for h in range(2):
                            lhsT = wb_sb[
                                :, (kw * 2 + h) * 128 : (kw * 2 + h) * 128 + 128
                            ]
                            for g in range(n_groups):
                                rhs = xgs[g][
                                    :, :,
                                    kw : kw + STRIDE * (nWp - 1) + 1 : STRIDE,
                                ].transpose([0, 2, 1])
                                nc.tensor.matmul(
                                    ps[g][h][:],
                                    lhsT,
                                    rhs,
                                    start=(kw == 0),
                                    stop=(kw == KW - 1),
                                )
                    for g in range(n_groups):
                        for h in range(2):
                            ob = op.tile([128, NF], _OUT_DTYPE, name="ob")
                            nc.vector.tensor_copy(ob[:], ps[g][h][:])
                            nc.scalar.dma_start(out_d[b, g, h], ob[:])

                # batched boundary remainder over all images
                psb = psbp.tile([MB, NB], f32, name=f"psb_{rep}", tag="psb")
                for ph in range(8):
                    rhsb = xb3[
                        :, :, ph : ph + STRIDE * (nWp - 1) + 1 : STRIDE
                    ].transpose([0, 2, 1])
                    nc.tensor.matmul(
                        psb[:],
                        wbb_sb[:, ph * MB : (ph + 1) * MB],
                        rhsb,
                        start=(ph == 0),
                        stop=(ph == 7),
                    )
                obb = op.tile([MB, NB], _OUT_DTYPE, name="obb")
                nc.vector.tensor_copy(obb[:], psb[:])
                nc.scalar.dma_start(outb_d[:], obb[:])

            if loop > 1:
                with tc.For_i(0, loop):
                    for rep in range(repeat):
                        emit_rep(rep)
            else:
                for rep in range(repeat):
                    emit_rep(rep)
    nc.compile()
    return nc, nH, nW, n_groups


def unscramble(dev_out, dev_outb, nH, nW, n_img):
    """Assemble [n_img, nH*4, nW*4] from the device layouts.

    dev_out  [n_img, n_groups, 2, 128, GROUP*nWp]: main results; split
             patches (i = 16t+15) hold only their kh<8 partial sums.
    dev_outb [16*n_bnd, n_img*nWp]: kh>=8 remainders for split patches.
    """
    n_groups = dev_out.shape[1]
    nWp = nW
    n_bnd = n_groups * GROUP - 1
    dev = np.asarray(dev_out, np.float32)
    dev = dev.reshape(n_img, n_groups, 2, 8, D0, D1, nWp, GROUP)
    bnd = np.asarray(dev_outb, np.float32)
    bnd = bnd.reshape(n_bnd, D0, D1, nWp, n_img)
    out5 = np.empty((n_img, nH, D0, nW, D1), np.float32)
    for g in range(n_groups):
        for t in range(GROUP):
            tau = g * GROUP + t
            for h in range(2):
                for il in range(8):
                    ig = il + 8 * h
                    i = SLOTS * tau + ig
                    if i >= nH:
                        continue
                    # dev[b, g, h, il, d0, d1, j, t] -> out5[b, i, d0, j, d1]
                    v = dev[:, g, h, il, :, :, :, t].transpose(0, 1, 3, 2)
                    if ig == SLOTS - 1:
                        # add the kh>=8 remainder: bnd[tau, d0, d1, j, b]
                        v = v + bnd[tau].transpose(3, 0, 2, 1)
                    out5[:, i] = v
    return out5.reshape(n_img, nH * D0, nW * D1)


def _pjrt_timed_run(nc, in_maps, n_iters=6):
    """Clone of bass2jax.run_bass_via_pjrt's multi-core path, without
    donation, with device-resident inputs, timing each execution.
    Returns (results_per_core, [wall_seconds per iter])."""
    import time

    import jax
    from jax.sharding import Mesh, PartitionSpec
    from jax.experimental.shard_map import shard_map

    from concourse import bass2jax, mybir as _mb
    from concourse.bass2jax import _bass_exec_p, partition_id_tensor

    bass2jax.install_neuronx_cc_hook()
    n_cores = len(in_maps)
    partition_name = nc.partition_id_tensor.name if nc.partition_id_tensor else None

    in_names, out_names, out_avals = [], [], []
    zero_outs = []
    for alloc in nc.m.functions[0].allocations:
        if not isinstance(alloc, _mb.MemoryLocationSet):
            continue
        name = alloc.memorylocations[0].name
        if alloc.kind == "ExternalInput":
            if name != partition_name:
                in_names.append(name)
        elif alloc.kind == "ExternalOutput":
            shape = tuple(alloc.tensor_shape)
            dtype = _mb.dt.np(alloc.dtype)
            out_names.append(name)
            out_avals.append(jax.core.ShapedArray(shape, dtype))
            zero_outs.append(np.zeros(shape, dtype))
    n_params = len(in_names)
    in_names_all = in_names + out_names
    if partition_name is not None:
        in_names_all.append(partition_name)

    def _body(*args):
        operands = list(args)
        if partition_name is not None:
            operands.append(partition_id_tensor())
        outs = _bass_exec_p.bind(
            *operands,
            out_avals=tuple(out_avals),
            in_names=tuple(in_names_all),
            out_names=tuple(out_names),
            lowering_input_output_aliases=(),
            sim_require_finite=True,
            sim_require_nnan=True,
            nc=nc,
        )
        return tuple(outs)

    devices = jax.devices()[:n_cores]
    mesh = Mesh(np.asarray(devices), ("core",))
    in_specs = (PartitionSpec("core"),) * (n_params + len(out_names))
    out_specs = (PartitionSpec("core"),) * len(out_names)
    sharded = jax.jit(
        shard_map(_body, mesh=mesh, in_specs=in_specs, out_specs=out_specs,
                  check_rep=False),
        keep_unused=True,
    )
    concat_in = [
        np.concatenate([np.asarray(in_maps[c][n]) for c in range(n_cores)], axis=0)
        for n in in_names
    ]
    concat_zeros = [
        np.zeros((n_cores * z.shape[0], *z.shape[1:]), z.dtype) for z in zero_outs
    ]
    from jax.sharding import NamedSharding

    dev_in = [
        jax.device_put(a, NamedSharding(mesh, PartitionSpec("core")))
        for a in concat_in + concat_zeros
    ]
    out_arrs = sharded(*dev_in)  # warmup + compile
    jax.block_until_ready(out_arrs)
    times = []
    for _ in range(n_iters):
        t0 = time.perf_counter()
        out_arrs = sharded(*dev_in)
        jax.block_until_ready(out_arrs)
        times.append(time.perf_counter() - t0)
    results = [
        {
            n: np.asarray(out_arrs[i]).reshape(n_cores, *out_avals[i].shape)[c]
            for i, n in enumerate(out_names)
        }
        for c in range(n_cores)
    ]
    return results, times


_CACHE = {}


def _get_nc(n_img, H, W, repeat=1, loop=1):
    key = (n_img, H, W, repeat, loop)
    if key not in _CACHE:
        _CACHE[key] = build_nc(n_img, H, W, repeat, loop)
    return _CACHE[key]


def kernel(x, weight, _timed=False, _repeat=1):
    x = np.asarray(x, np.float32)
    weight = np.asarray(weight, np.float32)
    B, H, W = x.shape
    assert B % N_CORES == 0
    n_img = B // N_CORES
    nc, nH, nW, n_groups = _get_nc(n_img, H, W, _repeat)
    wb = build_wband(weight)
    wbb = build_wband_bnd(weight, H // 128 - 1)
    in_maps = [
        {
            "x": np.ascontiguousarray(x[c * n_img : (c + 1) * n_img]).reshape(-1),
            "wb": wb,
            "wbb": wbb,
        }
        for c in range(N_CORES)
    ]
    if _timed:
        results, times = _pjrt_timed_run(nc, in_maps)
    else:
        results = run_bass_kernel_spmd(
            nc, in_maps, core_ids=list(range(N_CORES))
        ).results
        times = None
    shards = [
        unscramble(results[c]["out"], results[c]["outb"], nH, nW, n_img)
        for c in range(N_CORES)
    ]
    full = np.concatenate(shards, axis=0)
    if _timed:
        return full, times
    return full


# revision 20
# speedup vs baseline: 778.5957x; 1.0950x over previous
"""Trainium2 Bass kernel for strided-conv-as-linear (nn_ConvNd_60851096649851).

Computation (see reference): x [B,1024,1024] f32, weight [16,256] f32.
16x16 windows at stride 8 -> 127x127 patches; per patch y = W @ flat(window)
(16 outputs), reshaped to a 4x4 tile of the [B,508,508] output.

Strategy: data-parallel over batch (4 images per core, 8 cores).

Per image: 8 DISJOINT 128-row window tiles (x is read from HBM exactly
once). Rows live on SBUF partitions (natural layout). For each window
tile, out[(i_l,o), j] = sum_kw Wband_kw[row, (i_l,o)]^T @ x[row, 8j+kw]
where Wband_kw is the banded weight (nonzero at row = 8*i_l + kh): 16
accumulating float32r matmuls, K=128, M=128 (i_l in [0,8) x 16 outputs
per half; two halves cover 16 patch slots/tile). Four tiles are batched
in the moving dim (N = 4*128 = 512) so float32r streams 1 cycle/column.

The moving AP is [j, t] with t innermost (size 4, even, satisfying the
fp32r even-moving constraint) so no pad-j column is needed: N = 4*127 =
508 and the loads read exactly W columns per row (no spill handling).

The 16th patch slot of each tile (i = 16t+15, rows 128t+120..128t+135)
crosses the tile boundary: its kh 0..7 rows live in tile t (handled by
the main h=1 matmul band), its kh 8..15 rows are the first 8 rows of
tile t+1. Those remainders are computed by ONE batched boundary matmul
chain per rep: two column-shifted copies of the 7 boundary bands are
stacked in K (K=112, M=112) so 8 phase-matmuls (N=4 images x 127) cover
all 16 kw; the host adds the two partials for those 7 patch rows.

All DMA rides the HWDGE rings (the original baseline put ~70 dma_starts
on the gpsimd SWDGE path whose Q7 descriptor emission at ~1.4us each
dominated the runtime): x loads on nc.sync (SP) as one uniform-window
access pattern per 4-tile group, weights + output stores on nc.scalar
(ACT). Outputs are stored as bf16 (halves store traffic; rel err ~2e-3
<< 2e-2 gate). Host prepares banded weights and unscrambles the device
layout into [B,508,508] f32.
"""

import os
import sys

sys.path.insert(0, "/opt/trn_rl_repo")
os.environ.setdefault("JAX_PLATFORMS", "cpu")

import numpy as np

import concourse.bass as bass  # noqa: F401
import concourse.tile as tile
from concourse import bacc, mybir
from concourse.ap import AP
from concourse.bass_utils import run_bass_kernel_spmd

N_CORES = 8
KH = KW = 16
STRIDE = 8
D0 = D1 = 4  # per-patch output tile
OC = 16  # outputs per patch = D0*D1
SLOTS = 16  # patch slots per 128-row tile (slot 15 is split across tiles)
GROUP = 4  # window tiles batched per matmul (moving dim)

_MM_DTYPE = mybir.dt.float32r
_OUT_DTYPE = mybir.dt.bfloat16


def build_wband(weight):
    """Banded weights: [128, KW*2*128] f32.

    wb[p, kw, h, m] = W[o, kh*16+kw] where p = 8*(i_l+8h)+kh, m = i_l*16+o,
    i_l in [0,8). Slot i_l+8h==15 keeps only its kh<8 rows (p<128); the
    kh>=8 remainder comes from the boundary matmul.
    """
    W4 = np.asarray(weight, np.float32).reshape(OC, KH, KW)
    wb = np.zeros((128, KW, 2, 128), np.float32)
    for h in range(2):
        for il in range(8):
            ig = il + 8 * h
            for kh in range(KH):
                p = 8 * ig + kh
                if p >= 128:
                    continue
                wb[p, :, h, il * OC : (il + 1) * OC] = W4[:, kh, :].T
    return np.ascontiguousarray(wb.reshape(128, KW * 2 * 128))


def build_wband_bnd(weight, n_bnd):
    """Boundary band weights: [16*n_bnd, 8*16*n_bnd] f32.

    The kh>=8 remainder of split patch tau (tau in 0..n_bnd-1), read
    against rows 0..7 of tile tau+1. The boundary contraction only needs
    8*n_bnd=56 rows, so TWO copies of the bands are stacked in K (the
    second reads the x bands shifted by one conv stride) and each of the
    8 phase-matmuls covers two kw values at once:
      rows 8*tau+q       : W[o, 8+q, phase]      (x col 8j+phase)
      rows 56 + 8*tau+q  : W[o, 8+q, phase+8]    (x col 8j+8+phase)
    """
    W4 = np.asarray(weight, np.float32).reshape(OC, KH, KW)
    half = 8 * n_bnd
    wbb = np.zeros((2 * half, 8, OC * n_bnd), np.float32)
    for tau in range(n_bnd):
        for q in range(8):
            wbb[8 * tau + q, :, tau * OC : (tau + 1) * OC] = \
                W4[:, 8 + q, 0:8].T
            wbb[half + 8 * tau + q, :, tau * OC : (tau + 1) * OC] = \
                W4[:, 8 + q, 8:16].T
    return np.ascontiguousarray(wbb.reshape(2 * half, 8 * OC * n_bnd))


def build_nc(n_img, H, W, repeat=1, loop=1):
    """Build the per-core Bass program. Returns compiled nc.

    repeat: python-unrolled copies of the whole per-core computation.
    loop: hardware For_i trip count around those copies (timing only).
    """
    nH = (H - KH) // STRIDE + 1
    nW = (W - KW) // STRIDE + 1
    n_tiles = H // 128  # disjoint tiles
    assert n_tiles % GROUP == 0, (n_tiles, GROUP)
    n_groups = n_tiles // GROUP
    n_bnd = n_tiles - 1  # split patches per image
    KB = 16 * n_bnd  # boundary contraction size (two phase-shifted copies)
    MB = OC * n_bnd  # boundary output partitions
    nWp = nW  # no pad-j: moving AP is [j, t] with t innermost (even)
    NF = GROUP * nWp  # moving free size per main matmul
    NB = n_img * nWp  # moving free size per boundary matmul
    WS = W  # no pad column -> no spill past the row
    assert NB <= 512 and NF <= 512

    nc = bacc.Bacc(
        "TRN2", target_bir_lowering=False, debug=False, num_devices=N_CORES
    )
    f32 = mybir.dt.float32
    x_d = nc.dram_tensor(
        "x", [n_img * H * W], _MM_DTYPE, kind="ExternalInput"
    ).ap()
    wb_d = nc.dram_tensor(
        "wb", [128, KW * 2 * 128], _MM_DTYPE, kind="ExternalInput"
    ).ap()
    wbb_d = nc.dram_tensor(
        "wbb", [KB, 8 * MB], _MM_DTYPE, kind="ExternalInput"
    ).ap()
    out_d = nc.dram_tensor(
        "out", [n_img, n_groups, 2, 128, NF], _OUT_DTYPE, kind="ExternalOutput"
    ).ap()
    outb_d = nc.dram_tensor(
        "outb", [MB, NB], _OUT_DTYPE, kind="ExternalOutput"
    ).ap()

    with tile.TileContext(nc) as tc:
        with (
            tc.tile_pool(name="wbp", bufs=1) as wbp,
            tc.tile_pool(name="xp", bufs=7) as xp,
            tc.tile_pool(name="xbp", bufs=1 if repeat == 1 else 2) as xbp,
            tc.tile_pool(name="psp", bufs=7, space="PSUM") as psp,
            tc.tile_pool(name="psbp", bufs=1, space="PSUM") as psbp,
            tc.tile_pool(name="op", bufs=8) as op,
        ):
            wb_sb = wbp.tile([128, KW * 2 * 128], _MM_DTYPE)
            wbb_sb = wbp.tile([KB, 8 * MB], _MM_DTYPE)
            # weights ride the ACT ring in chunks so kw=0 matmuls can
            # start while later chunks stream
            for c in range(4):
                nc.scalar.dma_start(
                    wb_sb[:, c * 1024 : (c + 1) * 1024],
                    wb_d[:, c * 1024 : (c + 1) * 1024],
                )
            nc.scalar.dma_start(wbb_sb[:], wbb_d[:])

            def emit_rep(rep):
                xb = xbp.tile([KB, n_img * WS], _MM_DTYPE, name="xb")
                xb3 = xb.rearrange("p (b w) -> p b w", b=n_img)
                for b in range(n_img):
                    xbase = b * H * W
                    xgs = []
                    for g in range(n_groups):
                        xg = xp.tile([128, GROUP * WS], _MM_DTYPE, name="xg")
                        xg3 = xg.rearrange("p (t w) -> p t w", t=GROUP)
                        t0 = g * GROUP
                        # one DMA per 4-tile group: uniform window AP
                        src = AP(
                            x_d.tensor,
                            xbase + 128 * t0 * W,
                            [[W, 128], [128 * W, GROUP], [1, WS]],
                        )
                        nc.sync.dma_start(xg3[:, :, :], src)
                        xgs.append(xg3)
                    # boundary bands: rows 128*(tau+1) .. +7, tau in 0..6;
                    # second K-copy shifted by one conv stride (8 cols)
                    srcb = AP(
                        x_d.tensor,
                        xbase + 128 * W,
                        [[128 * W, n_bnd], [W, 8], [1, WS]],
                    )
                    nc.scalar.dma_start(xb3[0 : KB // 2, b, :], srcb)
                    srcb2 = AP(
                        x_d.tensor,
                        xbase + 128 * W + STRIDE,
                        [[128 * W, n_bnd], [W, 8], [1, WS]],
                    )
                    nc.scalar.dma_start(xb3[KB // 2 : KB, b, :], srcb2)

                    ps = [
                        [
                            psp.tile([128, NF], f32, name=f"ps_{b}_{g}_{h}",
                                     tag="ps")
                            for h in range(2)
                        ]
                        for g in range(n_groups)
                    ]
                    for kw in range(KW):
                        for h in range(2):
                            lhsT = wb_sb[
                                :, (kw * 2 + h) * 128 : (kw * 2 + h) * 128 + 128
                            ]
                            for g in range(n_groups):
                                rhs = xgs[g][
                                    :, :,
                                    kw : kw + STRIDE * (nWp - 1) + 1 : STRIDE,
                                ].transpose([0, 2, 1])
                                nc.tensor.matmul(
                                    ps[g][h][:],
                                    lhsT,
                                    rhs,
                                    start=(kw == 0),
                                    stop=(kw == KW - 1),
                                )
                    for g in range(n_groups):
                        for h in range(2):
                            ob = op.tile([128, NF], _OUT_DTYPE, name="ob")
                            nc.vector.tensor_copy(ob[:], ps[g][h][:])
                            nc.scalar.dma_start(out_d[b, g, h], ob[:])

                # batched boundary remainder over all images
                psb = psbp.tile([MB, NB], f32, name=f"psb_{rep}", tag="psb")
                for ph in range(8):
                    rhsb = xb3[
                        :, :, ph : ph + STRIDE * (nWp - 1) + 1 : STRIDE
                    ].transpose([0, 2, 1])
                    nc.tensor.matmul(
                        psb[:],
                        wbb_sb[:, ph * MB : (ph + 1) * MB],
                        rhsb,
                        start=(ph == 0),
                        stop=(ph == 7),
                    )
                obb = op.tile([MB, NB], _OUT_DTYPE, name="obb")
                nc.vector.tensor_copy(obb[:], psb[:])
                nc.scalar.dma_start(outb_d[:], obb[:])

            if loop > 1:
                with tc.For_i(0, loop):
                    for rep in range(repeat):
                        emit_rep(rep)
            else:
                for rep in range(repeat):
                    emit_rep(rep)
    nc.compile()
    return nc, nH, nW, n_groups


def unscramble(dev_out, dev_outb, nH, nW, n_img):
    """Assemble [n_img, nH*4, nW*4] from the device layouts.

    dev_out  [n_img, n_groups, 2, 128, GROUP*nWp]: main results; split
             patches (i = 16t+15) hold only their kh<8 partial sums.
    dev_outb [16*n_bnd, n_img*nWp]: kh>=8 remainders for split patches.
    """
    n_groups = dev_out.shape[1]
    nWp = nW
    n_bnd = n_groups * GROUP - 1
    dev = np.asarray(dev_out, np.float32)
    dev = dev.reshape(n_img, n_groups, 2, 8, D0, D1, nWp, GROUP)
    bnd = np.asarray(dev_outb, np.float32)
    bnd = bnd.reshape(n_bnd, D0, D1, nWp, n_img)
    out5 = np.empty((n_img, nH, D0, nW, D1), np.float32)
    for g in range(n_groups):
        for t in range(GROUP):
            tau = g * GROUP + t
            for h in range(2):
                for il in range(8):
                    ig = il + 8 * h
                    i = SLOTS * tau + ig
                    if i >= nH:
                        continue
                    # dev[b, g, h, il, d0, d1, j, t] -> out5[b, i, d0, j, d1]
                    v = dev[:, g, h, il, :, :, :, t].transpose(0, 1, 3, 2)
                    if ig == SLOTS - 1:
                        # add the kh>=8 remainder: bnd[tau, d0, d1, j, b]
                        v = v + bnd[tau].transpose(3, 0, 2, 1)
                    out5[:, i] = v
    return out5.reshape(n_img, nH * D0, nW * D1)


def _pjrt_timed_run(nc, in_maps, n_iters=6):
    """Clone of bass2jax.run_bass_via_pjrt's multi-core path, without
    donation, with device-resident inputs, timing each execution.
    Returns (results_per_core, [wall_seconds per iter])."""
    import time

    import jax
    from jax.sharding import Mesh, PartitionSpec
    from jax.experimental.shard_map import shard_map

    from concourse import bass2jax, mybir as _mb
    from concourse.bass2jax import _bass_exec_p, partition_id_tensor

    bass2jax.install_neuronx_cc_hook()
    n_cores = len(in_maps)
    partition_name = nc.partition_id_tensor.name if nc.partition_id_tensor else None

    in_names, out_names, out_avals = [], [], []
    zero_outs = []
    for alloc in nc.m.functions[0].allocations:
        if not isinstance(alloc, _mb.MemoryLocationSet):
            continue
        name = alloc.memorylocations[0].name
        if alloc.kind == "ExternalInput":
            if name != partition_name:
                in_names.append(name)
        elif alloc.kind == "ExternalOutput":
            shape = tuple(alloc.tensor_shape)
            dtype = _mb.dt.np(alloc.dtype)
            out_names.append(name)
            out_avals.append(jax.core.ShapedArray(shape, dtype))
            zero_outs.append(np.zeros(shape, dtype))
    n_params = len(in_names)
    in_names_all = in_names + out_names
    if partition_name is not None:
        in_names_all.append(partition_name)

    def _body(*args):
        operands = list(args)
        if partition_name is not None:
            operands.append(partition_id_tensor())
        outs = _bass_exec_p.bind(
            *operands,
            out_avals=tuple(out_avals),
            in_names=tuple(in_names_all),
            out_names=tuple(out_names),
            lowering_input_output_aliases=(),
            sim_require_finite=True,
            sim_require_nnan=True,
            nc=nc,
        )
        return tuple(outs)

    devices = jax.devices()[:n_cores]
    mesh = Mesh(np.asarray(devices), ("core",))
    in_specs = (PartitionSpec("core"),) * (n_params + len(out_names))
    out_specs = (PartitionSpec("core"),) * len(out_names)
    sharded = jax.jit(
        shard_map(_body, mesh=mesh, in_specs=in_specs, out_specs=out_specs,
                  check_rep=False),
        keep_unused=True,
    )
    concat_in = [
        np.concatenate([np.asarray(in_maps[c][n]) for c in range(n_cores)], axis=0)
        for n in in_names
    ]
    concat_zeros = [
        np.zeros((n_cores * z.shape[0], *z.shape[1:]), z.dtype) for z in zero_outs
    ]
    from jax.sharding import NamedSharding

    dev_in = [
        jax.device_put(a, NamedSharding(mesh, PartitionSpec("core")))
        for a in concat_in + concat_zeros
    ]
    out_arrs = sharded(*dev_in)  # warmup + compile
    jax.block_until_ready(out_arrs)
    times = []
    for _ in range(n_iters):
        t0 = time.perf_counter()
        out_arrs = sharded(*dev_in)
        jax.block_until_ready(out_arrs)
        times.append(time.perf_counter() - t0)
    results = [
        {
            n: np.asarray(out_arrs[i]).reshape(n_cores, *out_avals[i].shape)[c]
            for i, n in enumerate(out_names)
        }
        for c in range(n_cores)
    ]
    return results, times


_CACHE = {}


def _get_nc(n_img, H, W, repeat=1, loop=1):
    key = (n_img, H, W, repeat, loop)
    if key not in _CACHE:
        _CACHE[key] = build_nc(n_img, H, W, repeat, loop)
    return _CACHE[key]


def kernel(x, weight, _timed=False, _repeat=1):
    x = np.asarray(x, np.float32)
    weight = np.asarray(weight, np.float32)
    B, H, W = x.shape
    assert B % N_CORES == 0
    n_img = B // N_CORES
    nc, nH, nW, n_groups = _get_nc(n_img, H, W, _repeat)
    wb = build_wband(weight)
    wbb = build_wband_bnd(weight, H // 128 - 1)
    in_maps = [
        {
            "x": np.ascontiguousarray(x[c * n_img : (c + 1) * n_img]).reshape(-1),
            "wb": wb,
            "wbb": wbb,
        }
        for c in range(N_CORES)
    ]
    if _timed:
        results, times = _pjrt_timed_run(nc, in_maps)
    else:
        results = run_bass_kernel_spmd(
            nc, in_maps, core_ids=list(range(N_CORES))
        ).results
        times = None
    shards = [
        unscramble(results[c]["out"], results[c]["outb"], nH, nW, n_img)
        for c in range(N_CORES)
    ]
    full = np.concatenate(shards, axis=0)
    if _timed:
        return full, times
    return full
